# revision 9
# baseline (speedup 1.0000x reference)
"""Trainium2 Bass kernel for nn_Grapher (GNN message passing block).

Strategy: pure data-parallel over batch B=64 -> 8 cores x 8 samples.
Per sample, the edge conv collapses algebraically:
  max_k relu(BN(W_ec @ [x_i; x_j - x_i]))
    = relu(A[:,n] + max_k B[:,idx[n,k]] + shift)
with A = (W1-W2)*se @ h, B = W2*se @ h, so only two 768x384x210 matmuls
plus a 9-neighbor gather-max instead of a 768x768x1890 matmul.
The KNN runs on a 210x210 cosine matrix via vector-engine max/max_index/
match_replace (top-8 + 9th).  Mean-over-K of the LoRA edge prompts
commutes with the 1x1 conv, and is computed with an adjacency one-hot
matmul.  All BN scales/shifts are folded into weights on the host.
"""

import sys
from contextlib import ExitStack

import numpy as np

sys.path.insert(0, "/opt/trn_rl_repo")

import ml_dtypes  # noqa: E402
import concourse.bass as bass  # noqa: E402
import concourse.bacc as bacc  # noqa: E402
import concourse.mybir as mybir  # noqa: E402
import concourse.tile as tile  # noqa: E402
from concourse import library_config  # noqa: E402
from concourse.masks import make_identity  # noqa: E402

F32 = mybir.dt.float32
BF16 = mybir.dt.bfloat16
U32 = mybir.dt.uint32
I16 = mybir.dt.int16
AF = mybir.ActivationFunctionType
ALU = mybir.AluOpType

B, C, H, W = 64, 384, 14, 14
R, P, K = 32, 14, 9
H1, N = 15, 210
HW = H * W          # 196
EPS = 1e-5
NCORES = 8
SPC = B // NCORES   # samples per core = 8
NPAIRS = SPC // 2   # 4
CCH = C // 128      # 3 c-chunks
C2 = 2 * C          # 768
C2CH = C2 // 128    # 6
NT = (128, 82)      # node chunks: 210 = 128 + 82
NEG = -1.0e30
GELU_AF = AF.Gelu
DEBUG_DUMPS = False

_CACHE = {}


def _ceil(a, b):
    return (a + b - 1) // b


def _build_nc():
    nc = bacc.Bacc(
        "TRN2", target_bir_lowering=False, debug=False,
        enable_asserts=False, num_devices=NCORES,
    )
    d = {}
    di = {
        "x_d": ([NPAIRS, 128, CCH, 2, HW], F32),
        "wfc1t": ([128, CCH, C], F32),
        "bias1": ([128, CCH], F32),
        "prom": ([128, CCH, P], F32),
        "wdownt": ([128, CCH, R], F32),
        "bdown": ([R, 1], F32),
        "gp": ([R, C], F32),
        "wat": ([128, CCH, C2], BF16),
        "wbt": ([128, CCH, C2], BF16),
        "shifte": ([128, C2CH], F32),
        "wfc2t": ([128, C2CH, C], BF16),
        "wupt": ([R, C], F32),
        "shifto": ([128, CCH], F32),
    }
    for name, (shape, dt) in di.items():
        d[name] = nc.dram_tensor(name, shape, dt, kind="ExternalInput").ap()
    d["y_d"] = nc.dram_tensor(
        "y_d", [NPAIRS, 128, CCH, 2, HW], F32, kind="ExternalOutput"
    ).ap()
    if DEBUG_DUMPS:
        dbg = {
            "dbg_hp": ([128, CCH, 2, N], F32),
            "dbg_lrp": ([R, 2, N], F32),
            "dbg_hbp": ([128, CCH, 2, N], F32),
            "dbg_gs": ([128, 2, N], F32),
            "dbg_i9": ([128, 2, 9], U32),
            "dbg_ap": ([128, 2, C2], BF16),
            "dbg_bp": ([128, 2, C2], BF16),
            "dbg_gt": ([128, K, C2], BF16),
            "dbg_am": ([128, 2, C2], BF16),
            "dbg_rt": ([128, C2CH, 2, N], BF16),
            "dbg_lmp": ([R, 2, N], F32),
        }
        for name, (shape, dt) in dbg.items():
            d[name] = nc.dram_tensor(name, shape, dt, kind="ExternalOutput").ap()
    return nc, d


def _build_program():
    nc, d = _build_nc()
    with tile.TileContext(nc) as tc:
        with ExitStack() as ctx:
            _emit(ctx, tc, nc, d)
    nc.compile()
    return nc


def _emit(ctx, tc, nc, d):
    nc.gpsimd.load_library(library_config.mlp)
    wp = ctx.enter_context(tc.tile_pool(name="weights", bufs=1))
    pp = ctx.enter_context(tc.tile_pool(name="pair", bufs=2))
    sp = ctx.enter_context(tc.tile_pool(name="samp", bufs=2))
    pmm = ctx.enter_context(tc.tile_pool(name="pmm", bufs=3, space="PSUM"))
    plm = ctx.enter_context(tc.tile_pool(name="plm", bufs=1, space="PSUM"))
    ptr = ctx.enter_context(tc.tile_pool(name="ptr", bufs=2, space="PSUM"))
    pab = ctx.enter_context(tc.tile_pool(name="pab", bufs=2, space="PSUM"))
    dp = ctx.enter_context(tc.tile_pool(name="dscratch", bufs=2, space="DRAM"))

    # ---- persistent weights ----
    def wload(name, shape, dt):
        t = wp.tile(shape, dt, name=name)
        nc.sync.dma_start(t[:], d[name])
        return t

    wfc1t = wload("wfc1t", [128, CCH, C], F32)
    bias1 = wload("bias1", [128, CCH], F32)
    prom = wload("prom", [128, CCH, P], F32)
    wdownt = wload("wdownt", [128, CCH, R], F32)
    bdown = wload("bdown", [R, 1], F32)
    gp = wload("gp", [R, C], F32)
    wat = wload("wat", [128, CCH, C2], BF16)
    wbt = wload("wbt", [128, CCH, C2], BF16)
    shifte = wload("shifte", [128, C2CH], F32)
    wfc2t = wload("wfc2t", [128, C2CH, C], BF16)
    wupt = wload("wupt", [R, C], F32)
    shifto = wload("shifto", [128, CCH], F32)

    identf = wp.tile([128, 128], F32, name="identf")
    make_identity(nc, identf[:, :])
    identb = wp.tile([128, 128], BF16, name="identb")
    nc.vector.tensor_copy(identb[:, :], identf[:, :])
    id08 = wp.tile([128, 128], F32, name="id08")
    nc.vector.tensor_scalar_mul(id08[:, :], identf[:, :], 0.8)
    ones = wp.tile([128, 1], F32, name="ones")
    nc.vector.memset(ones[:, :], 1.0)
    z32 = wp.tile([48, 9], U32, name="z32")
    nc.vector.memset(z32[:, :], 0)

    for pair in range(NPAIRS):
        _emit_pair(tc, nc, d, pair, locals())


def _emit_pair(tc, nc, d, pair, env):
    pp, sp, pmm, plm, ptr, pab, dp = (env[k] for k in ("pp", "sp", "pmm", "plm", "ptr", "pab", "dp"))
    wfc1t, bias1, prom, wdownt, bdown, gp = (
        env[k] for k in ("wfc1t", "bias1", "prom", "wdownt", "bdown", "gp"))
    wat, wbt, shifte, wfc2t, wupt, shifto = (
        env[k] for k in ("wat", "wbt", "shifte", "wfc2t", "wupt", "shifto"))
    identf, identb, id08, ones = (env[k] for k in ("identf", "identb", "id08", "ones"))

    # ---- load x pair ----
    xp = pp.tile([128, CCH, 2, HW], F32, tag="xp")
    nc.sync.dma_start(xp[:], d["x_d"][pair])

    # ---- fc1 (+BN fold) : h_raw [c, n] per sample ----
    hp = pp.tile([128, CCH, 2, N], F32, tag="hp")
    for jo in range(CCH):
        ps = pmm.tile([128, 2, HW], F32, tag="mm")
        for ji in range(CCH):
            nc.tensor.matmul(
                out=ps[:, :, :],
                lhsT=wfc1t[:, ji, jo * 128:(jo + 1) * 128],
                rhs=xp[:, ji, :, :],
                start=(ji == 0), stop=(ji == CCH - 1),
            )
        for s2 in range(2):
            nc.scalar.activation(
                hp[:, jo, s2, :HW], ps[:, s2, :], AF.Identity,
                bias=bias1[:, jo:jo + 1],
            )
    for s2 in range(2):
        nc.scalar.activation(hp[:, :, s2, HW:N], prom[:, :, :], AF.Copy)

    # ---- LoRA down + gelu : lr [r, n] ----
    lrp = pp.tile([R, 2, N], F32, tag="lrp")
    psl = pmm.tile([R, 2, N], F32, tag="mm")
    for ji in range(CCH):
        nc.tensor.matmul(
            out=psl[:, :, :], lhsT=wdownt[:, ji, :], rhs=hp[:, ji, :, :],
            start=(ji == 0), stop=(ji == CCH - 1),
        )
    nc.scalar.activation(lrp[:, :, :], psl[:, :, :], GELU_AF, bias=bdown[:, 0:1])

    # ---- blend: hb = 0.8*h + 0.2*gp^T @ lr  (both f32 and bf16 copies) ----
    hbp = pp.tile([128, CCH, 2, N], F32, tag="hbp")
    hbb = pp.tile([128, CCH, 2, N], BF16, tag="hbb")
    for jo in range(CCH):
        ps = pmm.tile([128, 2, N], F32, tag="mm")
        nc.tensor.matmul(out=ps[:, :, :], lhsT=gp[:, jo * 128:(jo + 1) * 128],
                         rhs=lrp[:, :, :], start=True, stop=False)
        nc.tensor.matmul(out=ps[:, :, :], lhsT=id08[:, :], rhs=hp[:, jo, :, :],
                         start=False, stop=True)
        nc.scalar.activation(hbp[:, jo, :, :], ps[:, :, :], AF.Copy)
        nc.vector.tensor_copy(hbb[:, jo, :, :], ps[:, :, :])

    # ---- column norms -> cinv ----
    hsq = pp.tile([128, CCH, 2, N], F32, tag="hsq")
    nc.scalar.activation(hsq[:, :, :, :], hbp[:, :, :, :], AF.Square)
    pss = pmm.tile([1, 2, N], F32, tag="mm")
    for ji in range(CCH):
        nc.tensor.matmul(out=pss[:, :, :], lhsT=ones[:, :], rhs=hsq[:, ji, :, :],
                         start=(ji == 0), stop=(ji == CCH - 1))

    if DEBUG_DUMPS and pair == 0:
        nc.sync.dma_start(d["dbg_hp"], hp[:])
        nc.sync.dma_start(d["dbg_lrp"], lrp[:])
        nc.sync.dma_start(d["dbg_hbp"], hbp[:])
    for s2 in range(2):
        _emit_sample(tc, nc, d, pair, s2, env, hp, lrp, hbp, hbb, pss)

    # ---- fc2 + ep (paired) ----
    reluT = env["_reluT"]
    lmp = env["_lmp"]
    for jo in range(CCH):
        ps = pmm.tile([128, 2, N], F32, tag="mm")
        for jc in range(C2CH):
            nc.tensor.matmul(
                out=ps[:, :, :], lhsT=wfc2t[:, jc, jo * 128:(jo + 1) * 128],
                rhs=reluT[:, jc, :, :], start=(jc == 0), stop=False,
            )
        nc.tensor.matmul(out=ps[:, :, :], lhsT=wupt[:, jo * 128:(jo + 1) * 128],
                         rhs=lmp[:, :, :], start=False, stop=True)
        tf = pp.tile([128, 2, HW], F32, tag="tf")
        nc.scalar.activation(tf[:, :, :], ps[:, :, :HW], AF.Identity,
                             bias=shifto[:, jo:jo + 1])
        yo = pp.tile([128, 2, HW], F32, tag="yo")
        nc.vector.tensor_add(yo[:, :, :], tf[:, :, :], xp[:, jo, :, :])
        nc.sync.dma_start(d["y_d"][pair, :, jo, :, :], yo[:, :, :])


def _emit_sample(tc, nc, d, pair, s2, env, hp, lrp, hbp, hbb, pss):
    pp, sp, pmm, plm, ptr, pab, dp = (env[k] for k in ("pp", "sp", "pmm", "plm", "ptr", "pab", "dp"))
    identf, identb = env["identf"], env["identb"]
    wat, wbt, shifte = env["wat"], env["wbt"], env["shifte"]

    # ---- cinv ----
    den = sp.tile([1, N], F32, tag="den")
    nc.scalar.activation(den[:, :], pss[:1, s2, :], AF.Sqrt)
    nc.vector.tensor_scalar_add(den[:, :], den[:, :], 1e-12)
    cinv = sp.tile([1, N], F32, tag="cinv")
    nc.vector.reciprocal(cinv[:, :], den[:, :])
    cbc = sp.tile([128, N], F32, tag="cbc")
    nc.gpsimd.partition_broadcast(cbc[:, :], cinv[:, :])

    # ---- xn = hb * cinv (column-normalized) ----
    xn = sp.tile([128, CCH, N], F32, tag="xn")
    for j in range(CCH):
        nc.vector.tensor_mul(xn[:, j, :], hbp[:, j, s2, :], cbc[:, :])

    # ---- G[n, m] = hb[:,n] . xn[:,m] ----
    gs = sp.tile([128, 2, N], F32, tag="gs")
    for i, ni in enumerate(NT):
        ps = pmm.tile([128, N], F32, tag="mm")
        for j in range(CCH):
            nc.tensor.matmul(
                out=ps[:ni, :],
                lhsT=hbp[:, j, s2, i * 128:i * 128 + ni],
                rhs=xn[:, j, :],
                start=(j == 0), stop=(j == CCH - 1),
            )
        nc.scalar.activation(gs[:ni, i, :], ps[:ni, :], AF.Copy)

    # ---- top-9 per row: top-8 (max/max_index) + 9th (match_replace) ----
    m8 = sp.tile([128, 2, 8], F32, tag="m8")
    i9 = sp.tile([128, 2, 9], U32, tag="i9")
    gm = sp.tile([128, 2, N], F32, tag="gm")
    m8b = sp.tile([128, 2, 8], F32, tag="m8b")
    i8b = sp.tile([128, 2, 8], U32, tag="i8b")
    adj = sp.tile([128, 2, N], F32, tag="adj")
    for i, ni in enumerate(NT):
        nc.vector.max(m8[:ni, i, :], gs[:ni, i, :])
        nc.vector.max_index(i9[:ni, i, 0:8], m8[:ni, i, :], gs[:ni, i, :])
        nc.vector.match_replace(gm[:ni, i, :], m8[:ni, i, :], gs[:ni, i, :], NEG)
        nc.vector.max(m8b[:ni, i, :], gm[:ni, i, :])
        nc.vector.max_index(i8b[:ni, i, :], m8b[:ni, i, :], gm[:ni, i, :])
        nc.vector.tensor_copy(i9[:ni, i, 8:9], i8b[:ni, i, 0:1])
        nc.vector.tensor_scalar(
            adj[:ni, i, :], gs[:ni, i, :], m8b[:ni, i, 0:1], None, op0=ALU.is_ge,
        )

    if DEBUG_DUMPS and pair == 0 and s2 == 0:
        nc.sync.dma_start(d["dbg_gs"], gs[:])
        nc.sync.dma_start(d["dbg_i9"], i9[:])
    # ---- A, B edge-conv halves (bf16), B -> DRAM for the gather ----
    Ap = sp.tile([128, 2, C2], BF16, tag="Ap")
    Bp = sp.tile([128, 2, C2], BF16, tag="Bp")
    bvd = dp.tile([N, C2], BF16, tag="bvd")
    for i, ni in enumerate(NT):
        for wt, dst in ((wat, Ap), (wbt, Bp)):
            for hf in range(2):
                ps = pab.tile([128, 384], F32, tag="ab")
                for j in range(CCH):
                    nc.tensor.matmul(
                        out=ps[:ni, :],
                        lhsT=hbb[:, j, s2, i * 128:i * 128 + ni],
                        rhs=wt[:, j, hf * 384:(hf + 1) * 384],
                        start=(j == 0), stop=(j == CCH - 1),
                    )
                nc.scalar.activation(
                    dst[:ni, i, hf * 384:(hf + 1) * 384], ps[:ni, :], AF.Copy)
        nc.sync.dma_start(bvd[i * 128:i * 128 + ni, :], Bp[:ni, i, :])

    # ---- wrapped int16 index list for dma_gather ----
    # j = 256k + n ordering => gathered row (n, k) lands at out[n%128, 2k + n//128, :]
    z32 = env["z32"]
    didx = dp.tile([256, K], U32, tag="didx")
    nc.sync.dma_start(didx[208:256, :], z32[:, :])
    nc.sync.dma_start(didx[0:128, :], i9[:, 0, :])
    nc.sync.dma_start(didx[128:210, :], i9[:82, 1, :])
    idxw = sp.tile([128, 3, 48], I16, tag="idxw")
    nc.vector.memset(idxw[:, :, :], 0)
    # idxw[p, t, 16a+b] = low16(didx[16b+p, 3t+a])
    src = didx[:, :].bitcast(I16).rearrange(
        "(b p) (t a two) -> p t a b two", b=16, p=16, t=3, a=3, two=2)
    dst = idxw[0:16, :, :].rearrange("p t (a b) -> p t a b", a=3, b=16)
    nc.sync.dma_start(dst, src[:, :, :, :, 0])
    # hardware Q7 cores each read their own 16-partition stripe of idxs:
    # replicate the wrapped block across all 8 groups (log2 doubling)
    for g in (16, 32, 64):
        nc.sync.dma_start(idxw[g:2 * g, :, :], idxw[0:g, :, :])

    # ---- gather all 9 neighbor rows of B via 3 dma_gather calls ----
    gt = sp.tile([128, 18, C2], BF16, tag="gt")
    for t in range(3):
        nidx = 722 if t == 2 else 768
        ns = (nidx + 15) // 16
        nc.gpsimd.dma_gather(
            out_ap=gt[:, 6 * t:6 * t + 6, :], in_ap=bvd[:, :],
            idxs_ap=idxw[:, t, :ns], num_idxs=nidx, num_idxs_reg=nidx,
            elem_size=C2,
        )
    if DEBUG_DUMPS and pair == 0 and s2 == 0:
        nc.sync.dma_start(d["dbg_gt"], gt[:])

    # ---- max over the 9 gathered rows, am = A + max_k B ----
    am = sp.tile([128, 2, C2], BF16, tag="am")
    gv = gt[:, :, :].rearrange("p (k i) c -> p k i c", k=K, i=2)
    for i, ni in enumerate(NT):
        t1 = sp.tile([128, 4, C2], BF16, tag="t1")
        t2 = sp.tile([128, 2, C2], BF16, tag="t2")
        nc.vector.tensor_tensor(out=t1[:ni, :, :], in0=gv[:ni, 0:4, i, :],
                                in1=gv[:ni, 4:8, i, :], op=ALU.max)
        nc.vector.tensor_tensor(out=t2[:ni, :, :], in0=t1[:ni, 0:2, :],
                                in1=t1[:ni, 2:4, :], op=ALU.max)
        nc.vector.tensor_tensor(out=t1[:ni, 0, :], in0=t2[:ni, 0, :],
                                in1=t2[:ni, 1, :], op=ALU.max)
        nc.vector.tensor_tensor(out=t2[:ni, 0, :], in0=t1[:ni, 0, :],
                                in1=gv[:ni, 8, i, :], op=ALU.max)
        # am = A + max_k B
        nc.vector.tensor_add(am[:ni, i, :], Ap[:ni, i, :], t2[:ni, 0, :])

    if DEBUG_DUMPS and pair == 0 and s2 == 0:
        nc.sync.dma_start(d["dbg_ap"], Ap[:])
        nc.sync.dma_start(d["dbg_bp"], Bp[:])
        nc.sync.dma_start(d["dbg_am"], am[:])
    # ---- transpose am -> [c, n], relu(+shift_e) ----
    if s2 == 0:
        env["_reluT"] = pp.tile([128, C2CH, 2, N], BF16, tag="reluT", name="reluT")
    reluT = env["_reluT"]
    for cc in range(C2CH):
        for i, ni in enumerate(NT):
            pt = ptr.tile([128, 128], BF16, tag="tr")
            nc.tensor.transpose(
                pt[:, :ni], am[:ni, i, cc * 128:(cc + 1) * 128], identb[:ni, :ni])
            nc.scalar.activation(
                reluT[:, cc, s2, i * 128:i * 128 + ni], pt[:, :ni], AF.Relu,
                bias=shifte[:, cc:cc + 1],
            )

    # ---- lr^T and Adj^T transposes, lr_mean = (lr @ Adj^T)/9 ----
    lrT = sp.tile([128, 2, R], F32, tag="lrT")
    adjT = sp.tile([128, 2, N], F32, tag="adjT")
    for i, ni in enumerate(NT):
        pt = ptr.tile([128, 128], F32, tag="tr")
        nc.tensor.transpose(
            pt[:ni, :R], lrp[:, s2, i * 128:i * 128 + ni], identf[:R, :R])
        nc.scalar.activation(lrT[:ni, i, :], pt[:ni, :R], AF.Copy)
    for io, nio in enumerate(NT):
        for ii, nii in enumerate(NT):
            pt = ptr.tile([128, 128], F32, tag="tr")
            nc.tensor.transpose(
                pt[:nio, :nii],
                adj[:nii, ii, io * 128:io * 128 + nio],
                identf[:nii, :nii],
            )
            nc.scalar.activation(
                adjT[:nio, io, ii * 128:ii * 128 + nii], pt[:nio, :nii], AF.Copy)

    if s2 == 0:
        env["_lmp"] = pp.tile([R, 2, N], F32, tag="lmp", name="lmp")
        env["_pslm"] = plm.tile([R, 2, N], F32, tag="lm", name="pslm")
    lmp, pslm = env["_lmp"], env["_pslm"]
    for i, ni in enumerate(NT):
        nc.tensor.matmul(
            out=pslm[:, s2, :], lhsT=lrT[:ni, i, :], rhs=adjT[:ni, i, :],
            start=(i == 0), stop=(i == 1),
        )
    nc.scalar.activation(lmp[:, s2, :], pslm[:, s2, :], AF.Copy, scale=1.0 / 9.0)
    if DEBUG_DUMPS and pair == 0 and s2 == 1:
        nc.sync.dma_start(d["dbg_rt"], reluT[:])
        nc.sync.dma_start(d["dbg_lmp"], lmp[:])


# ======================= host side =======================

def _prep_inputs(inputs):
    f32 = np.float32
    bf = ml_dtypes.bfloat16
    s1 = (inputs["bn1_g"] / np.sqrt(inputs["bn1_v"] + EPS)).astype(f32)
    Wfc1 = (inputs["w_fc1"] * s1[:, None]).astype(f32)
    b1 = ((inputs["b_fc1"] - inputs["bn1_m"]) * s1 + inputs["bn1_b"]).astype(f32)
    se = (inputs["bne_g"] / np.sqrt(inputs["bne_v"] + EPS)).astype(f32)
    W1 = inputs["w_ec"][:, :C]
    W2 = inputs["w_ec"][:, C:]
    WA = ((W1 - W2) * se[:, None]).astype(f32)
    WB = (W2 * se[:, None]).astype(f32)
    shift_e = ((inputs["b_ec"] - inputs["bne_m"]) * se + inputs["bne_b"]).astype(f32)
    s2 = (inputs["bn2_g"] / np.sqrt(inputs["bn2_v"] + EPS)).astype(f32)
    Wfc2 = (0.8 * inputs["w_fc2"] * s2[:, None]).astype(f32)
    wup = (0.2 * inputs["w_up"]).astype(f32)
    shift_out = (0.8 * ((inputs["b_fc2"] - inputs["bn2_m"]) * s2 + inputs["bn2_b"])
                 + 0.2 * inputs["b_up"]).astype(f32)

    def chunk_pj(a, nch):  # [nch*128, ...] -> [128, nch, ...]
        return np.ascontiguousarray(
            a.reshape(nch, 128, *a.shape[1:]).transpose(1, 0, *range(2, a.ndim + 1)))

    w = {
        "wfc1t": chunk_pj(Wfc1.T.copy(), CCH),                  # [128,3,384]
        "bias1": chunk_pj(b1, CCH),                             # [128,3]
        "prom": chunk_pj(inputs["node_prompts"].astype(f32), CCH),
        "wdownt": chunk_pj(inputs["w_down"].T.astype(f32).copy(), CCH),
        "bdown": inputs["b_down"].astype(f32).reshape(R, 1),
        "gp": (0.2 * inputs["graph_prompt"]).astype(f32),       # [32,384]
        "wat": chunk_pj(WA.T.copy(), CCH).astype(bf),           # [128,3,768]
        "wbt": chunk_pj(WB.T.copy(), CCH).astype(bf),
        "shifte": chunk_pj(shift_e, C2CH),                      # [128,6]
        "wfc2t": chunk_pj(Wfc2.T.copy(), C2CH).astype(bf),      # [128,6,384]
        "wupt": wup.T.astype(f32).copy(),                       # [32,384]
        "shifto": chunk_pj(shift_out, CCH),                     # [128,3]
    }
    w = {k: np.ascontiguousarray(v) for k, v in w.items()}
    return w


def _shard_x(x):
    # -> per-core [NPAIRS, 128, CCH, 2, HW] f32
    shards = []
    for c in range(NCORES):
        xs = x[c * SPC:(c + 1) * SPC].reshape(SPC, C, HW)
        xs = xs.reshape(NPAIRS, 2, CCH, 128, HW).transpose(0, 3, 2, 1, 4)
        shards.append(np.ascontiguousarray(xs.astype(np.float32)))
    return shards


def _unshard_y(results):
    out = np.empty((B, C, H, W), np.float32)
    for c in range(NCORES):
        y = results[c]["y_d"]  # [NPAIRS,128,CCH,2,HW]
        ys = y.transpose(0, 3, 2, 1, 4).reshape(SPC, C, H, W)
        out[c * SPC:(c + 1) * SPC] = ys
    return out


def get_program():
    if "nc" not in _CACHE:
        _CACHE["nc"] = _build_program()
    return _CACHE["nc"]


def run(inputs, trace=False, **kw):
    from concourse.bass_utils import run_bass_kernel_spmd
    nc = get_program()
    w = _prep_inputs(inputs)
    shards = _shard_x(np.asarray(inputs["x"], np.float32))
    in_maps = [{**w, "x_d": shards[c]} for c in range(NCORES)]
    res = run_bass_kernel_spmd(nc, in_maps, list(range(NCORES)), trace=trace, **kw)
    return _unshard_y(res.results), res


def kernel(**inputs):
    y, _ = run(inputs)
    return y


if __name__ == "__main__":
    get_program()
    print("program built OK")



# revision 15
# speedup vs baseline: 1.0151x; 1.0151x over previous
"""Trainium2 Bass kernel for nn_Grapher (GNN message passing block).

Strategy: pure data-parallel over batch B=64 -> 8 cores x 8 samples.
Per sample the edge conv collapses algebraically:
  max_k relu(BN(W_ec @ [x_i; x_j - x_i]))
    = relu(A[:,n] + max_k B[:,idx[n,k]] + shift)
with A = (W1-W2)*se @ h, B = W2*se @ h.  The KNN runs on a 210x210
cosine matrix via vector-engine max/max_index/match_replace.  The
9-neighbor gather of B rows goes through DRAM with InstDMAGatherAnt
(3 calls/sample, wrapped int16 indices replicated across the 8 Q7
cores); index order j = 256k + n lands row (n,k) at out[n%128,
2k + n//128, :] so the max tree runs on strided slices.  Mean-over-K
of the LoRA edge prompts commutes with the 1x1 conv and uses an
adjacency one-hot matmul.  BN scales/shifts are folded on the host.

Pipeline: 2 blocks x 2 pairs; stage A (fc1/lora/blend/norms) batched
per block so Gelu/Sqrt activation-table loads cluster; stage B skewed
(B1 = gram/top9/idx/AB/gathers, B2 = tree/transposes) to hide gather
DMA latency; stage C (fc2+ep) per pair.  fc1/lora/blend run in f32r
(1 PE pass); gram/norms stay f32 to keep the KNN ranking exact.
"""

import sys
from contextlib import ExitStack

import numpy as np

sys.path.insert(0, "/opt/trn_rl_repo")

import ml_dtypes  # noqa: E402
import concourse.bass as bass  # noqa: E402
import concourse.bacc as bacc  # noqa: E402
import concourse.mybir as mybir  # noqa: E402
import concourse.tile as tile  # noqa: E402
from concourse import library_config  # noqa: E402
from concourse.masks import make_identity  # noqa: E402

F32 = mybir.dt.float32
F32R = mybir.dt.float32r
USE_F32R = False
BF16 = mybir.dt.bfloat16
U32 = mybir.dt.uint32
I16 = mybir.dt.int16
AF = mybir.ActivationFunctionType
ALU = mybir.AluOpType

B, C, H, W = 64, 384, 14, 14
R, P, K = 32, 14, 9
H1, N = 15, 210
HW = H * W          # 196
EPS = 1e-5
NCORES = 8
SPC = B // NCORES   # samples per core = 8
NPAIRS = SPC // 2   # 4
CCH = C // 128      # 3 c-chunks
C2 = 2 * C          # 768
C2CH = C2 // 128    # 6
NT = (128, 82)      # node chunks: 210 = 128 + 82
NEG = -1.0e30
GELU_AF = AF.Gelu

_CACHE = {}


def _maybe_r(ap):
    return ap.bitcast(F32R) if USE_F32R else ap


def _build_nc():
    nc = bacc.Bacc(
        "TRN2", target_bir_lowering=False, debug=False,
        enable_asserts=False, num_devices=NCORES,
    )
    d = {}
    di = {
        "x_d": ([NPAIRS, 128, CCH, 2, HW], F32),
        "wfc1t": ([128, CCH, C], F32),
        "bias1": ([128, CCH], F32),
        "prom": ([128, CCH, P], F32),
        "wdownt": ([128, CCH, R], F32),
        "bdown": ([R, 1], F32),
        "gp": ([R, C], F32),
        "wat": ([128, CCH, C2], BF16),
        "wbt": ([128, CCH, C2], BF16),
        "shifte": ([128, C2CH], F32),
        "wfc2t": ([128, C2CH, C], BF16),
        "wupt": ([R, C], BF16),
        "shifto": ([128, CCH], F32),
    }
    for name, (shape, dt) in di.items():
        d[name] = nc.dram_tensor(name, shape, dt, kind="ExternalInput").ap()
    d["y_d"] = nc.dram_tensor(
        "y_d", [NPAIRS, 128, CCH, 2, HW], F32, kind="ExternalOutput"
    ).ap()
    return nc, d


def _build_program():
    nc, d = _build_nc()
    with tile.TileContext(nc) as tc:
        with ExitStack() as ctx:
            Emitter(ctx, tc, nc, d).emit()
    nc.compile()
    return nc


class Emitter:
    def __init__(self, ctx, tc, nc, d):
        self.ctx, self.tc, self.nc, self.d = ctx, tc, nc, d
        self.pc = {}   # per-pair tile context: pc[pair] = dict

    def emit(self):
        ctx, tc, nc, d = self.ctx, self.tc, self.nc, self.d
        nc.gpsimd.load_library(library_config.mlp)
        self.wp = ctx.enter_context(tc.tile_pool(name="weights", bufs=1))
        self.pa = ctx.enter_context(tc.tile_pool(name="pairp", bufs=1))
        self.hp_pool = ctx.enter_context(tc.tile_pool(name="hptr", bufs=1))
        self.sp = ctx.enter_context(tc.tile_pool(name="samp", bufs=2))
        self.pmm = ctx.enter_context(tc.tile_pool(name="pmm", bufs=2, space="PSUM"))
        self.pab = ctx.enter_context(tc.tile_pool(name="pab", bufs=2, space="PSUM"))
        self.dp = ctx.enter_context(tc.tile_pool(name="dscratch", bufs=2, space="DRAM"))

        wp = self.wp

        def wload(name, shape, dt):
            t = wp.tile(shape, dt, name=name)
            nc.sync.dma_start(t[:], d[name])
            return t

        self.wfc1t = wload("wfc1t", [128, CCH, C], F32)
        self.bias1 = wload("bias1", [128, CCH], F32)
        self.prom = wload("prom", [128, CCH, P], F32)
        self.wdownt = wload("wdownt", [128, CCH, R], F32)
        self.bdown = wload("bdown", [R, 1], F32)
        self.gp = wload("gp", [R, C], F32)
        self.wat = wload("wat", [128, CCH, C2], BF16)
        self.wbt = wload("wbt", [128, CCH, C2], BF16)
        self.shifte = wload("shifte", [128, C2CH], F32)
        self.wfc2t = wload("wfc2t", [128, C2CH, C], BF16)
        self.wupt = wload("wupt", [R, C], BF16)
        self.shifto = wload("shifto", [128, CCH], F32)

        identf = wp.tile([128, 128], F32, name="identf")
        make_identity(nc, identf[:, :])
        self.identf = identf
        self.identb = wp.tile([128, 128], BF16, name="identb")
        nc.vector.tensor_copy(self.identb[:, :], identf[:, :])
        self.id08 = wp.tile([128, 128], F32, name="id08")
        nc.vector.tensor_scalar_mul(self.id08[:, :], identf[:, :], 0.8)
        self.ones = wp.tile([128, 1], F32, name="ones")
        nc.vector.memset(self.ones[:, :], 1.0)
        self.z32 = wp.tile([48, K], U32, name="z32")
        nc.vector.memset(self.z32[:, :], 0)

        for blk in range(2):
            pairs = (2 * blk, 2 * blk + 1)
            for q, pr in enumerate(pairs):
                self.stage_a1(pr, q)
            for q, pr in enumerate(pairs):
                self.stage_a2(pr, q)
            samples = [(pairs[0], 0), (pairs[0], 1), (pairs[1], 0), (pairs[1], 1)]
            prev = None
            for s in samples:
                self.stage_b1(s)
                if prev is not None:
                    self.stage_b2(prev)
                prev = s
            self.stage_b2(prev)
            for q, pr in enumerate(pairs):
                self.stage_c(pr, q)

    # ---- stage A1: fc1 + lora(Gelu) + blend ----
    def stage_a1(self, pr, q):
        nc, d = self.nc, self.d
        pc = self.pc[pr] = {}
        xp = self.pa.tile([128, CCH, 2, HW], F32, tag=f"xp{q}")
        nc.sync.dma_start(xp[:], d["x_d"][pr])
        pc["xp"] = xp

        hp = self.hp_pool.tile([128, CCH, 2, N], F32, tag=f"hp{q}")
        for jo in range(CCH):
            ps = self.pmm.tile([128, 2, HW], F32, tag="mm")
            for ji in range(CCH):
                nc.tensor.matmul(
                    out=ps[:, :, :],
                    lhsT=_maybe_r(self.wfc1t[:, ji, jo * 128:(jo + 1) * 128]),
                    rhs=_maybe_r(xp[:, ji, :, :]),
                    start=(ji == 0), stop=(ji == CCH - 1),
                )
            for s2 in range(2):
                nc.scalar.activation(
                    hp[:, jo, s2, :HW], ps[:, s2, :], AF.Identity,
                    bias=self.bias1[:, jo:jo + 1],
                )
        for s2 in range(2):
            nc.scalar.activation(hp[:, :, s2, HW:N], self.prom[:, :, :], AF.Copy)

        lrp = self.pa.tile([R, 2, N], F32, tag=f"lrp{q}")
        psl = self.pmm.tile([R, 2, N], F32, tag="mm")
        for ji in range(CCH):
            nc.tensor.matmul(
                out=psl[:, :, :], lhsT=_maybe_r(self.wdownt[:, ji, :]),
                rhs=_maybe_r(hp[:, ji, :, :]),
                start=(ji == 0), stop=(ji == CCH - 1),
            )
        nc.scalar.activation(lrp[:, :, :], psl[:, :, :], GELU_AF,
                             bias=self.bdown[:, 0:1])
        pc["lrp"] = lrp
        lrb = self.pa.tile([R, 2, N], BF16, tag=f"lrb{q}")
        nc.vector.tensor_copy(lrb[:, :, :], lrp[:, :, :])
        pc["lrb"] = lrb

        hbp = self.pa.tile([128, CCH, 2, N], F32, tag=f"hbp{q}")
        hbb = self.pa.tile([128, CCH, 2, N], BF16, tag=f"hbb{q}")
        for jo in range(CCH):
            ps = self.pmm.tile([128, 2, N], F32, tag="mm")
            nc.tensor.matmul(out=ps[:, :, :],
                             lhsT=_maybe_r(self.gp[:, jo * 128:(jo + 1) * 128]),
                             rhs=_maybe_r(lrp[:, :, :]), start=True, stop=False)
            nc.tensor.matmul(out=ps[:, :, :], lhsT=_maybe_r(self.id08[:, :]),
                             rhs=_maybe_r(hp[:, jo, :, :]),
                             start=False, stop=True)
            nc.scalar.activation(hbp[:, jo, :, :], ps[:, :, :], AF.Copy)
            nc.vector.tensor_copy(hbb[:, jo, :, :], ps[:, :, :])
        pc["hbp"], pc["hbb"] = hbp, hbb

        # reluT / lmp tiles persist until stage C
        reluT = self.pa.tile([128, C2CH, 2, N], BF16, tag=f"reluT{q}")
        lmp = self.pa.tile([R, 2, N], BF16, tag=f"lmp{q}")
        pc["reluT"], pc["lmp"] = reluT, lmp

    # ---- stage A2: column norms + cinv (Sqrt clustered per block) ----
    def stage_a2(self, pr, q):
        nc = self.nc
        pc = self.pc[pr]
        hbp = pc["hbp"]
        hsq = self.hp_pool.tile([128, CCH, 2, N], F32, tag=f"hsq{q}")
        nc.vector.tensor_mul(hsq[:, :, :, :], hbp[:, :, :, :], hbp[:, :, :, :])
        pss = self.pmm.tile([1, 2, N], F32, tag="mm")
        for ji in range(CCH):
            nc.tensor.matmul(out=pss[:, :, :], lhsT=self.ones[:, :],
                             rhs=hsq[:, ji, :, :],
                             start=(ji == 0), stop=(ji == CCH - 1))
        den = self.hp_pool.tile([1, 2, N], F32, tag=f"den{q}")
        nc.scalar.activation(den[:, :, :], pss[:, :, :], AF.Sqrt)
        nc.vector.tensor_scalar_add(den[:, :, :], den[:, :, :], 1e-12)
        cinv = self.pa.tile([1, 2, N], F32, tag=f"cinv{q}")
        nc.vector.reciprocal(cinv[:, :, :], den[:, :, :])
        pc["cinv"] = cinv

    # ---- stage B1: gram -> top9 -> idx chain -> A/B -> bvd -> gathers ----
    def stage_b1(self, s):
        pr, s2 = s
        nc, sp, dp = self.nc, self.sp, self.dp
        pc = self.pc[pr]
        hbp, hbb, cinv = pc["hbp"], pc["hbb"], pc["cinv"]

        cbc = sp.tile([128, N], F32, tag="cbc")
        nc.gpsimd.partition_broadcast(cbc[:, :], cinv[:1, s2, :])
        xn = sp.tile([128, CCH, N], F32, tag="xn")
        for j in range(CCH):
            nc.vector.tensor_mul(xn[:, j, :], hbp[:, j, s2, :], cbc[:, :])

        # G[n, m] = hb[:,n] . xn[:,m]  (f32: KNN ranking accuracy)
        gs = sp.tile([128, 2, N], F32, tag="gs")
        for i, ni in enumerate(NT):
            ps = self.pmm.tile([128, N], F32, tag="mm")
            for j in range(CCH):
                nc.tensor.matmul(
                    out=ps[:ni, :],
                    lhsT=hbp[:, j, s2, i * 128:i * 128 + ni],
                    rhs=xn[:, j, :],
                    start=(j == 0), stop=(j == CCH - 1),
                )
            nc.vector.tensor_copy(gs[:ni, i, :], ps[:ni, :])

        # top-9 per row: top-8 (max/max_index) + 9th (match_replace)
        m8 = sp.tile([128, 2, 8], F32, tag="m8")
        i9 = sp.tile([128, 2, K], U32, tag="i9")
        gm = sp.tile([128, 2, N], F32, tag="gm")
        m8b = sp.tile([128, 2, 8], F32, tag="m8b")
        i8b = sp.tile([128, 2, 8], U32, tag="i8b")
        adj = sp.tile([128, 2, N], BF16, tag="adj")
        for i, ni in enumerate(NT):
            nc.vector.max(m8[:ni, i, :], gs[:ni, i, :])
            nc.vector.max_index(i9[:ni, i, 0:8], m8[:ni, i, :], gs[:ni, i, :])
            nc.vector.match_replace(gm[:ni, i, :], m8[:ni, i, :], gs[:ni, i, :], NEG)
            nc.vector.max(m8b[:ni, i, :], gm[:ni, i, :])
            nc.vector.max_index(i8b[:ni, i, :], m8b[:ni, i, :], gm[:ni, i, :])
            nc.vector.tensor_copy(i9[:ni, i, 8:9], i8b[:ni, i, 0:1])
            nc.vector.tensor_scalar(
                adj[:ni, i, :], gs[:ni, i, :], m8b[:ni, i, 0:1], None, op0=ALU.is_ge,
            )
        pc[("i9", s2)] = i9
        pc[("adj", s2)] = adj

        # wrapped int16 index list: j = 256k + n -> (n,k) at out[n%128, 2k+n//128]
        didx = dp.tile([256, K], U32, tag="didx")
        nc.scalar.dma_start(didx[208:256, :], self.z32[:, :])
        nc.scalar.dma_start(didx[0:128, :], i9[:, 0, :])
        nc.scalar.dma_start(didx[128:210, :], i9[:82, 1, :])
        idxw = sp.tile([128, 3, 48], I16, tag="idxw")
        nc.vector.memset(idxw[:, :, :], 0)
        src = didx[:, :].bitcast(I16).rearrange(
            "(b p) (t a two) -> p t a b two", b=16, p=16, t=3, a=3, two=2)
        dst = idxw[0:16, :, :].rearrange("p t (a b) -> p t a b", a=3, b=16)
        nc.scalar.dma_start(dst, src[:, :, :, :, 0])
        # Q7 cores read their own 16-partition stripe: replicate (log2 doubling)
        for g in (16, 32, 64):
            nc.scalar.dma_start(idxw[g:2 * g, :, :], idxw[0:g, :, :])

        # A, B edge-conv halves (bf16), B -> DRAM for the gather
        ABp = sp.tile([128, 2, 2, C2], BF16, tag="ABp")
        bvd = dp.tile([N, C2], BF16, tag="bvd")
        for i, ni in enumerate(NT):
            for hf in range(2):
                ps = self.pab.tile([128, 2, 512], F32, tag="ab")
                for j in range(CCH):
                    lhs = hbb[:, j, s2, i * 128:i * 128 + ni]
                    nc.tensor.matmul(
                        out=ps[:ni, 0, 0:384], lhsT=lhs,
                        rhs=self.wat[:, j, hf * 384:(hf + 1) * 384],
                        start=(j == 0), stop=(j == CCH - 1),
                    )
                    nc.tensor.matmul(
                        out=ps[:ni, 1, 0:384], lhsT=lhs,
                        rhs=self.wbt[:, j, hf * 384:(hf + 1) * 384],
                        start=(j == 0), stop=(j == CCH - 1),
                    )
                nc.scalar.activation(
                    ABp[:ni, :, i, hf * 384:(hf + 1) * 384], ps[:ni, :, 0:384],
                    AF.Copy)
            nc.sync.dma_start(bvd[i * 128:i * 128 + ni, :], ABp[:ni, 1, i, :])
        pc[("ABp", s2)] = ABp

        gt = sp.tile([128, 18, C2], BF16, tag="gt")
        for t in range(3):
            nidx = 722 if t == 2 else 768
            ns = (nidx + 15) // 16
            nc.gpsimd.dma_gather(
                out_ap=gt[:, 6 * t:6 * t + 6, :], in_ap=bvd[:, :],
                idxs_ap=idxw[:, t, :ns], num_idxs=nidx, num_idxs_reg=nidx,
                elem_size=C2,
            )
        pc[("gt", s2)] = gt

    # ---- stage B2: max tree -> amT -> reluT; adjT/lrT -> lmp ----
    def stage_b2(self, s):
        pr, s2 = s
        nc, sp = self.nc, self.sp
        pc = self.pc[pr]
        gt, ABp = pc[("gt", s2)], pc[("ABp", s2)]
        i9, adj = pc[("i9", s2)], pc[("adj", s2)]
        reluT, lmp = pc["reluT"], pc["lmp"]
        lrb = pc["lrb"]

        am = sp.tile([128, 2, C2], BF16, tag="am")
        gv = gt[:, :, :].rearrange("p (k i) c -> p k i c", k=K, i=2)
        for i, ni in enumerate(NT):
            # in-place tree inside gt's k slots
            nc.vector.tensor_tensor(out=gv[:ni, 0:4, i, :], in0=gv[:ni, 0:4, i, :],
                                    in1=gv[:ni, 4:8, i, :], op=ALU.max)
            nc.vector.tensor_tensor(out=gv[:ni, 0:2, i, :], in0=gv[:ni, 0:2, i, :],
                                    in1=gv[:ni, 2:4, i, :], op=ALU.max)
            nc.vector.tensor_tensor(out=gv[:ni, 0, i, :], in0=gv[:ni, 0, i, :],
                                    in1=gv[:ni, 1, i, :], op=ALU.max)
            nc.vector.tensor_tensor(out=gv[:ni, 0, i, :], in0=gv[:ni, 0, i, :],
                                    in1=gv[:ni, 8, i, :], op=ALU.max)
            nc.vector.tensor_add(am[:ni, i, :], ABp[:ni, 0, i, :], gv[:ni, 0, i, :])

        # transpose am -> [c, n]; relu(+shift_e) -> reluT
        for cc in range(C2CH):
            pt = self.pab.tile([128, N], BF16, tag="tr")
            for i, ni in enumerate(NT):
                nc.tensor.transpose(
                    pt[:, i * 128:i * 128 + ni], am[:ni, i, cc * 128:(cc + 1) * 128],
                    self.identb[:ni, :ni])
            nc.scalar.activation(
                reluT[:, cc, s2, :], pt[:, :], AF.Relu,
                bias=self.shifte[:, cc:cc + 1],
            )

        # lr^T and Adj^T (bf16), lr_mean = (lr @ Adj^T)/9
        lrT = sp.tile([128, 2, R], BF16, tag="lrT")
        adjT = sp.tile([128, 2, N], BF16, tag="adjT")
        for i, ni in enumerate(NT):
            pt = self.pab.tile([128, N], BF16, tag="tr")
            nc.tensor.transpose(
                pt[:ni, :R], lrb[:, s2, i * 128:i * 128 + ni], self.identb[:R, :R])
            nc.scalar.activation(lrT[:ni, i, :], pt[:ni, :R], AF.Copy)
        for io, nio in enumerate(NT):
            pt = self.pab.tile([128, N], BF16, tag="tr")
            for ii, nii in enumerate(NT):
                nc.tensor.transpose(
                    pt[:nio, ii * 128:ii * 128 + nii],
                    adj[:nii, ii, io * 128:io * 128 + nio],
                    self.identb[:nii, :nii],
                )
            nc.scalar.activation(adjT[:nio, io, :], pt[:nio, :], AF.Copy)

        pslm = self.pmm.tile([R, N], F32, tag="mm")
        for i, ni in enumerate(NT):
            nc.tensor.matmul(
                out=pslm[:, :], lhsT=lrT[:ni, i, :], rhs=adjT[:ni, i, :],
                start=(i == 0), stop=(i == 1),
            )
        nc.scalar.activation(lmp[:, s2, :], pslm[:, :], AF.Copy, scale=1.0 / 9.0)

    # ---- stage C: fc2 + ep, residual, store ----
    def stage_c(self, pr, q):
        nc, d = self.nc, self.d
        pc = self.pc[pr]
        reluT, lmp, xp = pc["reluT"], pc["lmp"], pc["xp"]
        for jo in range(CCH):
            ps = self.pmm.tile([128, 2, N], F32, tag="mm")
            for jc in range(C2CH):
                nc.tensor.matmul(
                    out=ps[:, :, :], lhsT=self.wfc2t[:, jc, jo * 128:(jo + 1) * 128],
                    rhs=reluT[:, jc, :, :], start=(jc == 0), stop=False,
                )
            nc.tensor.matmul(out=ps[:, :, :],
                             lhsT=self.wupt[:, jo * 128:(jo + 1) * 128],
                             rhs=lmp[:, :, :], start=False, stop=True)
            tf = self.sp.tile([128, 2, HW], F32, tag="tf")
            nc.scalar.activation(tf[:, :, :], ps[:, :, :HW], AF.Identity,
                                 bias=self.shifto[:, jo:jo + 1])
            yo = self.sp.tile([128, 2, HW], F32, tag="yo")
            nc.vector.tensor_add(yo[:, :, :], tf[:, :, :], xp[:, jo, :, :])
            nc.sync.dma_start(d["y_d"][pr, :, jo, :, :], yo[:, :, :])


# ======================= host side =======================

def _prep_inputs(inputs):
    f32 = np.float32
    bf = ml_dtypes.bfloat16
    s1 = (inputs["bn1_g"] / np.sqrt(inputs["bn1_v"] + EPS)).astype(f32)
    Wfc1 = (inputs["w_fc1"] * s1[:, None]).astype(f32)
    b1 = ((inputs["b_fc1"] - inputs["bn1_m"]) * s1 + inputs["bn1_b"]).astype(f32)
    se = (inputs["bne_g"] / np.sqrt(inputs["bne_v"] + EPS)).astype(f32)
    W1 = inputs["w_ec"][:, :C]
    W2 = inputs["w_ec"][:, C:]
    WA = ((W1 - W2) * se[:, None]).astype(f32)
    WB = (W2 * se[:, None]).astype(f32)
    shift_e = ((inputs["b_ec"] - inputs["bne_m"]) * se + inputs["bne_b"]).astype(f32)
    s2 = (inputs["bn2_g"] / np.sqrt(inputs["bn2_v"] + EPS)).astype(f32)
    Wfc2 = (0.8 * inputs["w_fc2"] * s2[:, None]).astype(f32)
    wup = (0.2 * inputs["w_up"]).astype(f32)
    shift_out = (0.8 * ((inputs["b_fc2"] - inputs["bn2_m"]) * s2 + inputs["bn2_b"])
                 + 0.2 * inputs["b_up"]).astype(f32)

    def chunk_pj(a, nch):  # [nch*128, ...] -> [128, nch, ...]
        return np.ascontiguousarray(
            a.reshape(nch, 128, *a.shape[1:]).transpose(1, 0, *range(2, a.ndim + 1)))

    w = {
        "wfc1t": chunk_pj(Wfc1.T.copy(), CCH),                  # [128,3,384]
        "bias1": chunk_pj(b1, CCH),                             # [128,3]
        "prom": chunk_pj(inputs["node_prompts"].astype(f32), CCH),
        "wdownt": chunk_pj(inputs["w_down"].T.astype(f32).copy(), CCH),
        "bdown": inputs["b_down"].astype(f32).reshape(R, 1),
        "gp": (0.2 * inputs["graph_prompt"]).astype(f32),       # [32,384]
        "wat": chunk_pj(WA.T.copy(), CCH).astype(bf),           # [128,3,768]
        "wbt": chunk_pj(WB.T.copy(), CCH).astype(bf),
        "shifte": chunk_pj(shift_e, C2CH),                      # [128,6]
        "wfc2t": chunk_pj(Wfc2.T.copy(), C2CH).astype(bf),      # [128,6,384]
        "wupt": wup.T.copy().astype(bf),                        # [32,384]
        "shifto": chunk_pj(shift_out, CCH),                     # [128,3]
    }
    w = {k: np.ascontiguousarray(v) for k, v in w.items()}
    return w


def _shard_x(x):
    # -> per-core [NPAIRS, 128, CCH, 2, HW] f32
    shards = []
    for c in range(NCORES):
        xs = x[c * SPC:(c + 1) * SPC].reshape(SPC, C, HW)
        xs = xs.reshape(NPAIRS, 2, CCH, 128, HW).transpose(0, 3, 2, 1, 4)
        shards.append(np.ascontiguousarray(xs.astype(np.float32)))
    return shards


def _unshard_y(results):
    out = np.empty((B, C, H, W), np.float32)
    for c in range(NCORES):
        y = results[c]["y_d"]  # [NPAIRS,128,CCH,2,HW]
        ys = y.transpose(0, 3, 2, 1, 4).reshape(SPC, C, H, W)
        out[c * SPC:(c + 1) * SPC] = ys
    return out


def get_program():
    if "nc" not in _CACHE:
        _CACHE["nc"] = _build_program()
    return _CACHE["nc"]


def run(inputs, trace=False, **kw):
    from concourse.bass_utils import run_bass_kernel_spmd
    nc = get_program()
    w = _prep_inputs(inputs)
    shards = _shard_x(np.asarray(inputs["x"], np.float32))
    in_maps = [{**w, "x_d": shards[c]} for c in range(NCORES)]
    res = run_bass_kernel_spmd(nc, in_maps, list(range(NCORES)), trace=trace, **kw)
    return _unshard_y(res.results), res


def kernel(**inputs):
    y, _ = run(inputs)
    return y


if __name__ == "__main__":
    get_program()
    print("program built OK")


# revision 16
# speedup vs baseline: 1.0991x; 1.0827x over previous
"""Trainium2 Bass kernel for nn_Grapher (GNN message passing block).

Strategy: pure data-parallel over batch B=64 -> 8 cores x 8 samples.
Per sample the edge conv collapses algebraically:
  max_k relu(BN(W_ec @ [x_i; x_j - x_i]))
    = relu(A[:,n] + max_k B[:,idx[n,k]] + shift)
with A = (W1-W2)*se @ h, B = W2*se @ h.  The KNN runs on a 210x210
cosine matrix via vector-engine max/max_index/match_replace.  The
9-neighbor gather of B rows goes through DRAM with InstDMAGatherAnt
(3 calls/sample, wrapped int16 indices replicated across the 8 Q7
cores); index order j = 256k + n lands row (n,k) at out[n%128,
2k + n//128, :] so the max tree runs on strided slices.  Mean-over-K
of the LoRA edge prompts commutes with the 1x1 conv and uses an
adjacency one-hot matmul.  BN scales/shifts are folded on the host.

Pipeline: 2 blocks x 2 pairs; stage A (fc1/lora/blend/norms) batched
per block so Gelu/Sqrt activation-table loads cluster; stage B skewed
(B1 = gram/top9/idx/AB/gathers, B2 = tree/transposes) to hide gather
DMA latency; stage C (fc2+ep) per pair.  fc1/lora/blend run in f32r
(1 PE pass); gram/norms stay f32 to keep the KNN ranking exact.
"""

import sys
from contextlib import ExitStack

import numpy as np

sys.path.insert(0, "/opt/trn_rl_repo")

import ml_dtypes  # noqa: E402
import concourse.bass as bass  # noqa: E402
import concourse.bacc as bacc  # noqa: E402
import concourse.mybir as mybir  # noqa: E402
import concourse.tile as tile  # noqa: E402
from concourse import library_config  # noqa: E402
from concourse.masks import make_identity  # noqa: E402

F32 = mybir.dt.float32
F32R = mybir.dt.float32r
USE_F32R = True
BF16 = mybir.dt.bfloat16
U32 = mybir.dt.uint32
I16 = mybir.dt.int16
AF = mybir.ActivationFunctionType
ALU = mybir.AluOpType

B, C, H, W = 64, 384, 14, 14
R, P, K = 32, 14, 9
H1, N = 15, 210
HW = H * W          # 196
EPS = 1e-5
NCORES = 8
SPC = B // NCORES   # samples per core = 8
NPAIRS = SPC // 2   # 4
CCH = C // 128      # 3 c-chunks
C2 = 2 * C          # 768
C2CH = C2 // 128    # 6
NT = (128, 82)      # node chunks: 210 = 128 + 82
NEG = -1.0e30
GELU_AF = AF.Gelu

_CACHE = {}


def _maybe_r(ap):
    return ap.bitcast(F32R) if USE_F32R else ap


def _build_nc():
    nc = bacc.Bacc(
        "TRN2", target_bir_lowering=False, debug=False,
        enable_asserts=False, num_devices=NCORES,
    )
    d = {}
    di = {
        "x_d": ([NPAIRS, 128, CCH, 2, HW], F32),
        "wfc1t": ([128, CCH, C], F32),
        "bias1": ([128, CCH], F32),
        "prom": ([128, CCH, P], F32),
        "wdownt": ([128, CCH, R], F32),
        "bdown": ([R, 1], F32),
        "gp": ([R, C], F32),
        "wat": ([128, CCH, C2], BF16),
        "wbt": ([128, CCH, C2], BF16),
        "shifte": ([128, C2CH], F32),
        "wfc2t": ([128, C2CH, C], BF16),
        "wupt": ([R, C], BF16),
        "shifto": ([128, CCH], F32),
    }
    for name, (shape, dt) in di.items():
        d[name] = nc.dram_tensor(name, shape, dt, kind="ExternalInput").ap()
    d["y_d"] = nc.dram_tensor(
        "y_d", [NPAIRS, 128, CCH, 2, HW], F32, kind="ExternalOutput"
    ).ap()
    return nc, d


def _build_program():
    nc, d = _build_nc()
    with tile.TileContext(nc) as tc:
        with ExitStack() as ctx:
            Emitter(ctx, tc, nc, d).emit()
    nc.compile()
    return nc


class Emitter:
    def __init__(self, ctx, tc, nc, d):
        self.ctx, self.tc, self.nc, self.d = ctx, tc, nc, d
        self.pc = {}   # per-pair tile context: pc[pair] = dict

    def emit(self):
        ctx, tc, nc, d = self.ctx, self.tc, self.nc, self.d
        nc.gpsimd.load_library(library_config.mlp)
        self.wp = ctx.enter_context(tc.tile_pool(name="weights", bufs=1))
        self.pa = ctx.enter_context(tc.tile_pool(name="pairp", bufs=1))
        self.hp_pool = ctx.enter_context(tc.tile_pool(name="hptr", bufs=1))
        self.sp = ctx.enter_context(tc.tile_pool(name="samp", bufs=2))
        self.pmm = ctx.enter_context(tc.tile_pool(name="pmm", bufs=2, space="PSUM"))
        self.pab = ctx.enter_context(tc.tile_pool(name="pab", bufs=2, space="PSUM"))
        self.dp = ctx.enter_context(tc.tile_pool(name="dscratch", bufs=2, space="DRAM"))

        wp = self.wp

        def wload(name, shape, dt):
            t = wp.tile(shape, dt, name=name)
            nc.sync.dma_start(t[:], d[name])
            return t

        self.wfc1t = wload("wfc1t", [128, CCH, C], F32)
        self.bias1 = wload("bias1", [128, CCH], F32)
        self.prom = wload("prom", [128, CCH, P], F32)
        self.wdownt = wload("wdownt", [128, CCH, R], F32)
        self.bdown = wload("bdown", [R, 1], F32)
        self.gp = wload("gp", [R, C], F32)
        self.wat = wload("wat", [128, CCH, C2], BF16)
        self.wbt = wload("wbt", [128, CCH, C2], BF16)
        self.shifte = wload("shifte", [128, C2CH], F32)
        self.wfc2t = wload("wfc2t", [128, C2CH, C], BF16)
        self.wupt = wload("wupt", [R, C], BF16)
        self.shifto = wload("shifto", [128, CCH], F32)

        identf = wp.tile([128, 128], F32, name="identf")
        make_identity(nc, identf[:, :])
        self.identf = identf
        self.identb = wp.tile([128, 128], BF16, name="identb")
        nc.vector.tensor_copy(self.identb[:, :], identf[:, :])
        self.id08 = wp.tile([128, 128], F32, name="id08")
        nc.vector.tensor_scalar_mul(self.id08[:, :], identf[:, :], 0.8)
        self.ones = wp.tile([128, 1], F32, name="ones")
        nc.vector.memset(self.ones[:, :], 1.0)
        self.z32 = wp.tile([48, K], U32, name="z32")
        nc.vector.memset(self.z32[:, :], 0)

        for blk in range(2):
            pairs = (2 * blk, 2 * blk + 1)
            for q, pr in enumerate(pairs):
                self.stage_a1(pr, q)
            for q, pr in enumerate(pairs):
                self.stage_a2(pr, q)
            samples = [(pairs[0], 0), (pairs[0], 1), (pairs[1], 0), (pairs[1], 1)]
            prev = None
            for s in samples:
                self.stage_b1(s)
                if prev is not None:
                    self.stage_b2(prev)
                prev = s
            self.stage_b2(prev)
            for q, pr in enumerate(pairs):
                self.stage_c(pr, q)

    # ---- stage A1: fc1 + lora(Gelu) + blend ----
    def stage_a1(self, pr, q):
        nc, d = self.nc, self.d
        pc = self.pc[pr] = {}
        xp = self.pa.tile([128, CCH, 2, HW], F32, tag=f"xp{q}")
        nc.sync.dma_start(xp[:], d["x_d"][pr])
        pc["xp"] = xp

        hp = self.hp_pool.tile([128, CCH, 2, N], F32, tag=f"hp{q}")
        for jo in range(CCH):
            ps = self.pmm.tile([128, 2, HW], F32, tag="mm")
            for ji in range(CCH):
                nc.tensor.matmul(
                    out=ps[:, :, :],
                    lhsT=_maybe_r(self.wfc1t[:, ji, jo * 128:(jo + 1) * 128]),
                    rhs=_maybe_r(xp[:, ji, :, :]),
                    start=(ji == 0), stop=(ji == CCH - 1),
                )
            for s2 in range(2):
                nc.scalar.activation(
                    hp[:, jo, s2, :HW], ps[:, s2, :], AF.Identity,
                    bias=self.bias1[:, jo:jo + 1],
                )
        for s2 in range(2):
            nc.scalar.activation(hp[:, :, s2, HW:N], self.prom[:, :, :], AF.Copy)

        lrp = self.pa.tile([R, 2, N], F32, tag=f"lrp{q}")
        psl = self.pmm.tile([R, 2, N], F32, tag="mm")
        for ji in range(CCH):
            nc.tensor.matmul(
                out=psl[:, :, :], lhsT=_maybe_r(self.wdownt[:, ji, :]),
                rhs=_maybe_r(hp[:, ji, :, :]),
                start=(ji == 0), stop=(ji == CCH - 1),
            )
        nc.scalar.activation(lrp[:, :, :], psl[:, :, :], GELU_AF,
                             bias=self.bdown[:, 0:1])
        pc["lrp"] = lrp
        lrb = self.pa.tile([R, 2, N], BF16, tag=f"lrb{q}")
        nc.vector.tensor_copy(lrb[:, :, :], lrp[:, :, :])
        pc["lrb"] = lrb

        hbp = self.pa.tile([128, CCH, 2, N], F32, tag=f"hbp{q}")
        hbb = self.pa.tile([128, CCH, 2, N], BF16, tag=f"hbb{q}")
        for jo in range(CCH):
            ps = self.pmm.tile([128, 2, N], F32, tag="mm")
            nc.tensor.matmul(out=ps[:, :, :],
                             lhsT=_maybe_r(self.gp[:, jo * 128:(jo + 1) * 128]),
                             rhs=_maybe_r(lrp[:, :, :]), start=True, stop=False)
            nc.tensor.matmul(out=ps[:, :, :], lhsT=_maybe_r(self.id08[:, :]),
                             rhs=_maybe_r(hp[:, jo, :, :]),
                             start=False, stop=True)
            nc.scalar.activation(hbp[:, jo, :, :], ps[:, :, :], AF.Copy)
            nc.vector.tensor_copy(hbb[:, jo, :, :], ps[:, :, :])
        pc["hbp"], pc["hbb"] = hbp, hbb

        # reluT / lmp tiles persist until stage C
        reluT = self.pa.tile([128, C2CH, 2, N], BF16, tag=f"reluT{q}")
        lmp = self.pa.tile([R, 2, N], BF16, tag=f"lmp{q}")
        pc["reluT"], pc["lmp"] = reluT, lmp

    # ---- stage A2: column norms + cinv (Sqrt clustered per block) ----
    def stage_a2(self, pr, q):
        nc = self.nc
        pc = self.pc[pr]
        hbp = pc["hbp"]
        hsq = self.hp_pool.tile([128, CCH, 2, N], F32, tag=f"hsq{q}")
        nc.vector.tensor_mul(hsq[:, :, :, :], hbp[:, :, :, :], hbp[:, :, :, :])
        pss = self.pmm.tile([1, 2, N], F32, tag="mm")
        for ji in range(CCH):
            nc.tensor.matmul(out=pss[:, :, :], lhsT=self.ones[:, :],
                             rhs=hsq[:, ji, :, :],
                             start=(ji == 0), stop=(ji == CCH - 1))
        den = self.hp_pool.tile([1, 2, N], F32, tag=f"den{q}")
        nc.scalar.activation(den[:, :, :], pss[:, :, :], AF.Sqrt)
        nc.vector.tensor_scalar_add(den[:, :, :], den[:, :, :], 1e-12)
        cinv = self.pa.tile([1, 2, N], F32, tag=f"cinv{q}")
        nc.vector.reciprocal(cinv[:, :, :], den[:, :, :])
        pc["cinv"] = cinv

    # ---- stage B1: gram -> top9 -> idx chain -> A/B -> bvd -> gathers ----
    def stage_b1(self, s):
        pr, s2 = s
        nc, sp, dp = self.nc, self.sp, self.dp
        pc = self.pc[pr]
        hbp, hbb, cinv = pc["hbp"], pc["hbb"], pc["cinv"]

        cbc = sp.tile([128, N], F32, tag="cbc")
        nc.gpsimd.partition_broadcast(cbc[:, :], cinv[:1, s2, :])
        xn = sp.tile([128, CCH, N], F32, tag="xn")
        for j in range(CCH):
            nc.vector.tensor_mul(xn[:, j, :], hbp[:, j, s2, :], cbc[:, :])

        # G[n, m] = hb[:,n] . xn[:,m]  (f32: KNN ranking accuracy)
        gs = sp.tile([128, 2, N], F32, tag="gs")
        for i, ni in enumerate(NT):
            ps = self.pmm.tile([128, N], F32, tag="mm")
            for j in range(CCH):
                nc.tensor.matmul(
                    out=ps[:ni, :],
                    lhsT=hbp[:, j, s2, i * 128:i * 128 + ni],
                    rhs=xn[:, j, :],
                    start=(j == 0), stop=(j == CCH - 1),
                )
            nc.vector.tensor_copy(gs[:ni, i, :], ps[:ni, :])

        # top-9 per row: top-8 (max/max_index) + 9th (match_replace)
        m8 = sp.tile([128, 2, 8], F32, tag="m8")
        i9 = sp.tile([128, 2, K], U32, tag="i9")
        gm = sp.tile([128, 2, N], F32, tag="gm")
        m8b = sp.tile([128, 2, 8], F32, tag="m8b")
        i8b = sp.tile([128, 2, 8], U32, tag="i8b")
        adj = sp.tile([128, 2, N], BF16, tag="adj")
        for i, ni in enumerate(NT):
            nc.vector.max(m8[:ni, i, :], gs[:ni, i, :])
            nc.vector.max_index(i9[:ni, i, 0:8], m8[:ni, i, :], gs[:ni, i, :])
            nc.vector.match_replace(gm[:ni, i, :], m8[:ni, i, :], gs[:ni, i, :], NEG)
            nc.vector.max(m8b[:ni, i, :], gm[:ni, i, :])
            nc.vector.max_index(i8b[:ni, i, :], m8b[:ni, i, :], gm[:ni, i, :])
            nc.vector.tensor_copy(i9[:ni, i, 8:9], i8b[:ni, i, 0:1])
            nc.vector.tensor_scalar(
                adj[:ni, i, :], gs[:ni, i, :], m8b[:ni, i, 0:1], None, op0=ALU.is_ge,
            )
        pc[("i9", s2)] = i9
        pc[("adj", s2)] = adj

        # wrapped int16 index list: j = 256k + n -> (n,k) at out[n%128, 2k+n//128]
        didx = dp.tile([256, K], U32, tag="didx")
        nc.scalar.dma_start(didx[208:256, :], self.z32[:, :])
        nc.scalar.dma_start(didx[0:128, :], i9[:, 0, :])
        nc.scalar.dma_start(didx[128:210, :], i9[:82, 1, :])
        idxw = sp.tile([128, 3, 48], I16, tag="idxw")
        nc.vector.memset(idxw[:, :, :], 0)
        src = didx[:, :].bitcast(I16).rearrange(
            "(b p) (t a two) -> p t a b two", b=16, p=16, t=3, a=3, two=2)
        dst = idxw[0:16, :, :].rearrange("p t (a b) -> p t a b", a=3, b=16)
        nc.scalar.dma_start(dst, src[:, :, :, :, 0])
        # Q7 cores read their own 16-partition stripe: replicate (log2 doubling)
        for g in (16, 32, 64):
            nc.scalar.dma_start(idxw[g:2 * g, :, :], idxw[0:g, :, :])

        # A, B edge-conv halves (bf16), B -> DRAM for the gather
        ABp = sp.tile([128, 2, 2, C2], BF16, tag="ABp")
        bvd = dp.tile([N, C2], BF16, tag="bvd")
        for i, ni in enumerate(NT):
            for hf in range(2):
                ps = self.pab.tile([128, 2, 512], F32, tag="ab")
                for j in range(CCH):
                    lhs = hbb[:, j, s2, i * 128:i * 128 + ni]
                    nc.tensor.matmul(
                        out=ps[:ni, 0, 0:384], lhsT=lhs,
                        rhs=self.wat[:, j, hf * 384:(hf + 1) * 384],
                        start=(j == 0), stop=(j == CCH - 1),
                    )
                    nc.tensor.matmul(
                        out=ps[:ni, 1, 0:384], lhsT=lhs,
                        rhs=self.wbt[:, j, hf * 384:(hf + 1) * 384],
                        start=(j == 0), stop=(j == CCH - 1),
                    )
                nc.scalar.activation(
                    ABp[:ni, :, i, hf * 384:(hf + 1) * 384], ps[:ni, :, 0:384],
                    AF.Copy)
            nc.sync.dma_start(bvd[i * 128:i * 128 + ni, :], ABp[:ni, 1, i, :])
        pc[("ABp", s2)] = ABp

        gt = sp.tile([128, 18, C2], BF16, tag="gt")
        for t in range(3):
            nidx = 722 if t == 2 else 768
            ns = (nidx + 15) // 16
            nc.gpsimd.dma_gather(
                out_ap=gt[:, 6 * t:6 * t + 6, :], in_ap=bvd[:, :],
                idxs_ap=idxw[:, t, :ns], num_idxs=nidx, num_idxs_reg=nidx,
                elem_size=C2,
            )
        pc[("gt", s2)] = gt

    # ---- stage B2: max tree -> amT -> reluT; adjT/lrT -> lmp ----
    def stage_b2(self, s):
        pr, s2 = s
        nc, sp = self.nc, self.sp
        pc = self.pc[pr]
        gt, ABp = pc[("gt", s2)], pc[("ABp", s2)]
        i9, adj = pc[("i9", s2)], pc[("adj", s2)]
        reluT, lmp = pc["reluT"], pc["lmp"]
        lrb = pc["lrb"]

        am = sp.tile([128, 2, C2], BF16, tag="am")
        gv = gt[:, :, :].rearrange("p (k i) c -> p k i c", k=K, i=2)
        for i, ni in enumerate(NT):
            # in-place tree inside gt's k slots
            nc.vector.tensor_tensor(out=gv[:ni, 0:4, i, :], in0=gv[:ni, 0:4, i, :],
                                    in1=gv[:ni, 4:8, i, :], op=ALU.max)
            nc.vector.tensor_tensor(out=gv[:ni, 0:2, i, :], in0=gv[:ni, 0:2, i, :],
                                    in1=gv[:ni, 2:4, i, :], op=ALU.max)
            nc.vector.tensor_tensor(out=gv[:ni, 0, i, :], in0=gv[:ni, 0, i, :],
                                    in1=gv[:ni, 1, i, :], op=ALU.max)
            nc.vector.tensor_tensor(out=gv[:ni, 0, i, :], in0=gv[:ni, 0, i, :],
                                    in1=gv[:ni, 8, i, :], op=ALU.max)
            nc.vector.tensor_add(am[:ni, i, :], ABp[:ni, 0, i, :], gv[:ni, 0, i, :])

        # transpose am -> [c, n]; relu(+shift_e) -> reluT
        for cc in range(C2CH):
            pt = self.pab.tile([128, N], BF16, tag="tr")
            for i, ni in enumerate(NT):
                nc.tensor.transpose(
                    pt[:, i * 128:i * 128 + ni], am[:ni, i, cc * 128:(cc + 1) * 128],
                    self.identb[:ni, :ni])
            nc.scalar.activation(
                reluT[:, cc, s2, :], pt[:, :], AF.Relu,
                bias=self.shifte[:, cc:cc + 1],
            )

        # lr^T and Adj^T (bf16), lr_mean = (lr @ Adj^T)/9
        lrT = sp.tile([128, 2, R], BF16, tag="lrT")
        adjT = sp.tile([128, 2, N], BF16, tag="adjT")
        for i, ni in enumerate(NT):
            pt = self.pab.tile([128, N], BF16, tag="tr")
            nc.tensor.transpose(
                pt[:ni, :R], lrb[:, s2, i * 128:i * 128 + ni], self.identb[:R, :R])
            nc.scalar.activation(lrT[:ni, i, :], pt[:ni, :R], AF.Copy)
        for io, nio in enumerate(NT):
            pt = self.pab.tile([128, N], BF16, tag="tr")
            for ii, nii in enumerate(NT):
                nc.tensor.transpose(
                    pt[:nio, ii * 128:ii * 128 + nii],
                    adj[:nii, ii, io * 128:io * 128 + nio],
                    self.identb[:nii, :nii],
                )
            nc.scalar.activation(adjT[:nio, io, :], pt[:nio, :], AF.Copy)

        pslm = self.pmm.tile([R, N], F32, tag="mm")
        for i, ni in enumerate(NT):
            nc.tensor.matmul(
                out=pslm[:, :], lhsT=lrT[:ni, i, :], rhs=adjT[:ni, i, :],
                start=(i == 0), stop=(i == 1),
            )
        nc.scalar.activation(lmp[:, s2, :], pslm[:, :], AF.Copy, scale=1.0 / 9.0)

    # ---- stage C: fc2 + ep, residual, store ----
    def stage_c(self, pr, q):
        nc, d = self.nc, self.d
        pc = self.pc[pr]
        reluT, lmp, xp = pc["reluT"], pc["lmp"], pc["xp"]
        for jo in range(CCH):
            ps = self.pmm.tile([128, 2, N], F32, tag="mm")
            for jc in range(C2CH):
                nc.tensor.matmul(
                    out=ps[:, :, :], lhsT=self.wfc2t[:, jc, jo * 128:(jo + 1) * 128],
                    rhs=reluT[:, jc, :, :], start=(jc == 0), stop=False,
                )
            nc.tensor.matmul(out=ps[:, :, :],
                             lhsT=self.wupt[:, jo * 128:(jo + 1) * 128],
                             rhs=lmp[:, :, :], start=False, stop=True)
            tf = self.sp.tile([128, 2, HW], F32, tag="tf")
            nc.scalar.activation(tf[:, :, :], ps[:, :, :HW], AF.Identity,
                                 bias=self.shifto[:, jo:jo + 1])
            yo = self.sp.tile([128, 2, HW], F32, tag="yo")
            nc.vector.tensor_add(yo[:, :, :], tf[:, :, :], xp[:, jo, :, :])
            nc.sync.dma_start(d["y_d"][pr, :, jo, :, :], yo[:, :, :])


# ======================= host side =======================

def _prep_inputs(inputs):
    f32 = np.float32
    bf = ml_dtypes.bfloat16
    s1 = (inputs["bn1_g"] / np.sqrt(inputs["bn1_v"] + EPS)).astype(f32)
    Wfc1 = (inputs["w_fc1"] * s1[:, None]).astype(f32)
    b1 = ((inputs["b_fc1"] - inputs["bn1_m"]) * s1 + inputs["bn1_b"]).astype(f32)
    se = (inputs["bne_g"] / np.sqrt(inputs["bne_v"] + EPS)).astype(f32)
    W1 = inputs["w_ec"][:, :C]
    W2 = inputs["w_ec"][:, C:]
    WA = ((W1 - W2) * se[:, None]).astype(f32)
    WB = (W2 * se[:, None]).astype(f32)
    shift_e = ((inputs["b_ec"] - inputs["bne_m"]) * se + inputs["bne_b"]).astype(f32)
    s2 = (inputs["bn2_g"] / np.sqrt(inputs["bn2_v"] + EPS)).astype(f32)
    Wfc2 = (0.8 * inputs["w_fc2"] * s2[:, None]).astype(f32)
    wup = (0.2 * inputs["w_up"]).astype(f32)
    shift_out = (0.8 * ((inputs["b_fc2"] - inputs["bn2_m"]) * s2 + inputs["bn2_b"])
                 + 0.2 * inputs["b_up"]).astype(f32)

    def chunk_pj(a, nch):  # [nch*128, ...] -> [128, nch, ...]
        return np.ascontiguousarray(
            a.reshape(nch, 128, *a.shape[1:]).transpose(1, 0, *range(2, a.ndim + 1)))

    w = {
        "wfc1t": chunk_pj(Wfc1.T.copy(), CCH),                  # [128,3,384]
        "bias1": chunk_pj(b1, CCH),                             # [128,3]
        "prom": chunk_pj(inputs["node_prompts"].astype(f32), CCH),
        "wdownt": chunk_pj(inputs["w_down"].T.astype(f32).copy(), CCH),
        "bdown": inputs["b_down"].astype(f32).reshape(R, 1),
        "gp": (0.2 * inputs["graph_prompt"]).astype(f32),       # [32,384]
        "wat": chunk_pj(WA.T.copy(), CCH).astype(bf),           # [128,3,768]
        "wbt": chunk_pj(WB.T.copy(), CCH).astype(bf),
        "shifte": chunk_pj(shift_e, C2CH),                      # [128,6]
        "wfc2t": chunk_pj(Wfc2.T.copy(), C2CH).astype(bf),      # [128,6,384]
        "wupt": wup.T.copy().astype(bf),                        # [32,384]
        "shifto": chunk_pj(shift_out, CCH),                     # [128,3]
    }
    w = {k: np.ascontiguousarray(v) for k, v in w.items()}
    return w


def _shard_x(x):
    # -> per-core [NPAIRS, 128, CCH, 2, HW] f32
    shards = []
    for c in range(NCORES):
        xs = x[c * SPC:(c + 1) * SPC].reshape(SPC, C, HW)
        xs = xs.reshape(NPAIRS, 2, CCH, 128, HW).transpose(0, 3, 2, 1, 4)
        shards.append(np.ascontiguousarray(xs.astype(np.float32)))
    return shards


def _unshard_y(results):
    out = np.empty((B, C, H, W), np.float32)
    for c in range(NCORES):
        y = results[c]["y_d"]  # [NPAIRS,128,CCH,2,HW]
        ys = y.transpose(0, 3, 2, 1, 4).reshape(SPC, C, H, W)
        out[c * SPC:(c + 1) * SPC] = ys
    return out


def get_program():
    if "nc" not in _CACHE:
        _CACHE["nc"] = _build_program()
    return _CACHE["nc"]


def run(inputs, trace=False, **kw):
    from concourse.bass_utils import run_bass_kernel_spmd
    nc = get_program()
    w = _prep_inputs(inputs)
    shards = _shard_x(np.asarray(inputs["x"], np.float32))
    in_maps = [{**w, "x_d": shards[c]} for c in range(NCORES)]
    res = run_bass_kernel_spmd(nc, in_maps, list(range(NCORES)), trace=trace, **kw)
    return _unshard_y(res.results), res


def kernel(**inputs):
    y, _ = run(inputs)
    return y


if __name__ == "__main__":
    get_program()
    print("program built OK")


# revision 18
# speedup vs baseline: 1.1422x; 1.0392x over previous
"""Trainium2 Bass kernel for nn_Grapher (GNN message passing block).

Strategy: pure data-parallel over batch B=64 -> 8 cores x 8 samples.
Per sample the edge conv collapses algebraically:
  max_k relu(BN(W_ec @ [x_i; x_j - x_i]))
    = relu(A[:,n] + max_k B[:,idx[n,k]] + shift)
with A = (W1-W2)*se @ h, B = W2*se @ h.  The KNN runs on a 210x210
cosine matrix via vector-engine max/max_index/match_replace.  The
9-neighbor gather of B rows goes through DRAM with InstDMAGatherAnt
(3 calls/sample, wrapped int16 indices replicated across the 8 Q7
cores); index order j = 256k + n lands row (n,k) at out[n%128,
2k + n//128, :] so the max tree runs on strided slices.  Mean-over-K
of the LoRA edge prompts commutes with the 1x1 conv and uses an
adjacency one-hot matmul.  BN scales/shifts are folded on the host.

Pipeline: 2 blocks x 2 pairs; stage A (fc1/lora/blend/norms) batched
per block so Gelu/Sqrt activation-table loads cluster; stage B skewed
(B1 = gram/top9/idx/AB/gathers, B2 = tree/transposes) to hide gather
DMA latency; stage C (fc2+ep) per pair.  fc1/lora/blend run in f32r
(1 PE pass); gram/norms stay f32 to keep the KNN ranking exact.
"""

import sys
from contextlib import ExitStack

import numpy as np

sys.path.insert(0, "/opt/trn_rl_repo")

import ml_dtypes  # noqa: E402
import concourse.bass as bass  # noqa: E402
import concourse.bacc as bacc  # noqa: E402
import concourse.mybir as mybir  # noqa: E402
import concourse.tile as tile  # noqa: E402
from concourse import library_config  # noqa: E402
from concourse.masks import make_identity  # noqa: E402

F32 = mybir.dt.float32
F32R = mybir.dt.float32r
USE_F32R = False
BF16 = mybir.dt.bfloat16
U32 = mybir.dt.uint32
I16 = mybir.dt.int16
AF = mybir.ActivationFunctionType
ALU = mybir.AluOpType

B, C, H, W = 64, 384, 14, 14
R, P, K = 32, 14, 9
H1, N = 15, 210
HW = H * W          # 196
EPS = 1e-5
NCORES = 8
SPC = B // NCORES   # samples per core = 8
NPAIRS = SPC // 2   # 4
CCH = C // 128      # 3 c-chunks
C2 = 2 * C          # 768
C2CH = C2 // 128    # 6
NT = (128, 82)      # node chunks: 210 = 128 + 82
NEG = -1.0e30
GELU_AF = AF.Gelu

_CACHE = {}


def _maybe_r(ap):
    return ap.bitcast(F32R) if USE_F32R else ap


def _build_nc():
    nc = bacc.Bacc(
        "TRN2", target_bir_lowering=False, debug=False,
        enable_asserts=False, num_devices=NCORES,
    )
    d = {}
    di = {
        "x_d": ([NPAIRS, 128, CCH, 2, HW], F32),
        "wfc1t": ([128, CCH, C], F32),
        "bias1": ([128, CCH], F32),
        "prom": ([128, CCH, P], F32),
        "wdownt": ([128, CCH, R], F32),
        "bdown": ([R, 1], F32),
        "gp": ([R, C], F32),
        "wat": ([128, CCH, C2], BF16),
        "wbt": ([128, CCH, C2], BF16),
        "shifte": ([128, C2CH], F32),
        "wfc2t": ([128, C2CH, C], BF16),
        "wupt": ([R, C], BF16),
        "shifto": ([128, CCH], F32),
    }
    for name, (shape, dt) in di.items():
        d[name] = nc.dram_tensor(name, shape, dt, kind="ExternalInput").ap()
    d["y_d"] = nc.dram_tensor(
        "y_d", [NPAIRS, 128, CCH, 2, HW], F32, kind="ExternalOutput"
    ).ap()
    return nc, d


def _build_program():
    nc, d = _build_nc()
    with tile.TileContext(nc) as tc:
        with ExitStack() as ctx:
            Emitter(ctx, tc, nc, d).emit()
    nc.compile()
    return nc


class Emitter:
    def __init__(self, ctx, tc, nc, d):
        self.ctx, self.tc, self.nc, self.d = ctx, tc, nc, d
        self.pc = {}   # per-pair tile context: pc[pair] = dict

    def emit(self):
        ctx, tc, nc, d = self.ctx, self.tc, self.nc, self.d
        nc.gpsimd.load_library(library_config.mlp)
        self.wp = ctx.enter_context(tc.tile_pool(name="weights", bufs=1))
        self.pa = ctx.enter_context(tc.tile_pool(name="pairp", bufs=1))
        self.hp_pool = ctx.enter_context(tc.tile_pool(name="hptr", bufs=1))
        self.sp = ctx.enter_context(tc.tile_pool(name="samp", bufs=3))
        self.gtp = ctx.enter_context(tc.tile_pool(name="gtp", bufs=3))
        self.pmm = ctx.enter_context(tc.tile_pool(name="pmm", bufs=2, space="PSUM"))
        self.pab = ctx.enter_context(tc.tile_pool(name="pab", bufs=2, space="PSUM"))
        self.dp = ctx.enter_context(tc.tile_pool(name="dscratch", bufs=3, space="DRAM"))

        wp = self.wp

        def wload(name, shape, dt):
            t = wp.tile(shape, dt, name=name)
            nc.sync.dma_start(t[:], d[name])
            return t

        self.wfc1t = wload("wfc1t", [128, CCH, C], F32)
        self.bias1 = wload("bias1", [128, CCH], F32)
        self.prom = wload("prom", [128, CCH, P], F32)
        self.wdownt = wload("wdownt", [128, CCH, R], F32)
        self.bdown = wload("bdown", [R, 1], F32)
        self.gp = wload("gp", [R, C], F32)
        self.wat = wload("wat", [128, CCH, C2], BF16)
        self.wbt = wload("wbt", [128, CCH, C2], BF16)
        self.shifte = wload("shifte", [128, C2CH], F32)
        self.wfc2t = wload("wfc2t", [128, C2CH, C], BF16)
        self.wupt = wload("wupt", [R, C], BF16)
        self.shifto = wload("shifto", [128, CCH], F32)

        identf = wp.tile([128, 128], F32, name="identf")
        make_identity(nc, identf[:, :])
        self.identf = identf
        self.identb = wp.tile([128, 128], BF16, name="identb")
        nc.vector.tensor_copy(self.identb[:, :], identf[:, :])
        self.id08 = wp.tile([128, 128], F32, name="id08")
        nc.vector.tensor_scalar_mul(self.id08[:, :], identf[:, :], 0.8)
        self.ones = wp.tile([128, 1], F32, name="ones")
        nc.vector.memset(self.ones[:, :], 1.0)
        self.z32 = wp.tile([48, K], U32, name="z32")
        nc.vector.memset(self.z32[:, :], 0)

        for blk in range(2):
            pairs = (2 * blk, 2 * blk + 1)
            for q, pr in enumerate(pairs):
                self.stage_a1(pr, q)
            for q, pr in enumerate(pairs):
                self.stage_a2(pr, q)
            samples = [(pairs[0], 0), (pairs[0], 1), (pairs[1], 0), (pairs[1], 1)]
            for si, s in enumerate(samples):
                self.stage_b1(s)
                if si >= 2:
                    self.stage_b2(samples[si - 2])
            self.stage_b2(samples[2])
            self.stage_b2(samples[3])
            for q, pr in enumerate(pairs):
                self.stage_c(pr, q)

    # ---- stage A1: fc1 + lora(Gelu) + blend ----
    def stage_a1(self, pr, q):
        nc, d = self.nc, self.d
        pc = self.pc[pr] = {}
        xp = self.pa.tile([128, CCH, 2, HW], F32, tag=f"xp{q}")
        nc.sync.dma_start(xp[:], d["x_d"][pr])
        pc["xp"] = xp

        hp = self.hp_pool.tile([128, CCH, 2, N], F32, tag=f"hp{q}")
        for jo in range(CCH):
            ps = self.pmm.tile([128, 2, HW], F32, tag="mm")
            for ji in range(CCH):
                nc.tensor.matmul(
                    out=ps[:, :, :],
                    lhsT=_maybe_r(self.wfc1t[:, ji, jo * 128:(jo + 1) * 128]),
                    rhs=_maybe_r(xp[:, ji, :, :]),
                    start=(ji == 0), stop=(ji == CCH - 1),
                )
            for s2 in range(2):
                nc.scalar.activation(
                    hp[:, jo, s2, :HW], ps[:, s2, :], AF.Identity,
                    bias=self.bias1[:, jo:jo + 1],
                )
        for s2 in range(2):
            nc.scalar.activation(hp[:, :, s2, HW:N], self.prom[:, :, :], AF.Copy)

        lrp = self.pa.tile([R, 2, N], F32, tag=f"lrp{q}")
        psl = self.pmm.tile([R, 2, N], F32, tag="mm")
        for ji in range(CCH):
            nc.tensor.matmul(
                out=psl[:, :, :], lhsT=_maybe_r(self.wdownt[:, ji, :]),
                rhs=_maybe_r(hp[:, ji, :, :]),
                start=(ji == 0), stop=(ji == CCH - 1),
            )
        nc.scalar.activation(lrp[:, :, :], psl[:, :, :], GELU_AF,
                             bias=self.bdown[:, 0:1])
        pc["lrp"] = lrp
        lrb = self.pa.tile([R, 2, N], BF16, tag=f"lrb{q}")
        nc.vector.tensor_copy(lrb[:, :, :], lrp[:, :, :])
        pc["lrb"] = lrb

        hbp = self.pa.tile([128, CCH, 2, N], F32, tag=f"hbp{q}")
        hbb = self.pa.tile([128, CCH, 2, N], BF16, tag=f"hbb{q}")
        for jo in range(CCH):
            ps = self.pmm.tile([128, 2, N], F32, tag="mm")
            nc.tensor.matmul(out=ps[:, :, :],
                             lhsT=_maybe_r(self.gp[:, jo * 128:(jo + 1) * 128]),
                             rhs=_maybe_r(lrp[:, :, :]), start=True, stop=False)
            nc.tensor.matmul(out=ps[:, :, :], lhsT=_maybe_r(self.id08[:, :]),
                             rhs=_maybe_r(hp[:, jo, :, :]),
                             start=False, stop=True)
            nc.scalar.activation(hbp[:, jo, :, :], ps[:, :, :], AF.Copy)
            nc.vector.tensor_copy(hbb[:, jo, :, :], ps[:, :, :])
        pc["hbp"], pc["hbb"] = hbp, hbb

        # reluT / lmp tiles persist until stage C
        reluT = self.pa.tile([128, C2CH, 2, N], BF16, tag=f"reluT{q}")
        lmp = self.pa.tile([R, 2, N], BF16, tag=f"lmp{q}")
        pc["reluT"], pc["lmp"] = reluT, lmp

    # ---- stage A2: column norms + cinv (Sqrt clustered per block) ----
    def stage_a2(self, pr, q):
        nc = self.nc
        pc = self.pc[pr]
        hbp = pc["hbp"]
        hsq = self.hp_pool.tile([128, CCH, 2, N], F32, tag=f"hsq{q}")
        nc.vector.tensor_mul(hsq[:, :, :, :], hbp[:, :, :, :], hbp[:, :, :, :])
        pss = self.pmm.tile([1, 2, N], F32, tag="mm")
        for ji in range(CCH):
            nc.tensor.matmul(out=pss[:, :, :], lhsT=self.ones[:, :],
                             rhs=hsq[:, ji, :, :],
                             start=(ji == 0), stop=(ji == CCH - 1))
        den = self.hp_pool.tile([1, 2, N], F32, tag=f"den{q}")
        nc.scalar.activation(den[:, :, :], pss[:, :, :], AF.Sqrt)
        nc.vector.tensor_scalar_add(den[:, :, :], den[:, :, :], 1e-12)
        cinv = self.pa.tile([1, 2, N], F32, tag=f"cinv{q}")
        nc.vector.reciprocal(cinv[:, :, :], den[:, :, :])
        pc["cinv"] = cinv

    # ---- stage B1: gram -> top9 -> idx chain -> A/B -> bvd -> gathers ----
    def stage_b1(self, s):
        pr, s2 = s
        nc, sp, dp = self.nc, self.sp, self.dp
        pc = self.pc[pr]
        hbp, hbb, cinv = pc["hbp"], pc["hbb"], pc["cinv"]

        cbc = sp.tile([128, N], F32, tag="cbc")
        nc.gpsimd.partition_broadcast(cbc[:, :], cinv[:1, s2, :])
        xn = sp.tile([128, CCH, N], F32, tag="xn")
        for j in range(CCH):
            nc.vector.tensor_mul(xn[:, j, :], hbp[:, j, s2, :], cbc[:, :])

        # G[n, m] = hb[:,n] . xn[:,m]  (f32: KNN ranking accuracy)
        gs = sp.tile([128, 2, N], F32, tag="gs")
        for i, ni in enumerate(NT):
            ps = self.pmm.tile([128, N], F32, tag="mm")
            for j in range(CCH):
                nc.tensor.matmul(
                    out=ps[:ni, :],
                    lhsT=hbp[:, j, s2, i * 128:i * 128 + ni],
                    rhs=xn[:, j, :],
                    start=(j == 0), stop=(j == CCH - 1),
                )
            nc.vector.tensor_copy(gs[:ni, i, :], ps[:ni, :])

        # top-9 per row: top-8 (max/max_index) + 9th (match_replace)
        m8 = sp.tile([128, 2, 8], F32, tag="m8")
        i9 = sp.tile([128, 2, K], U32, tag="i9")
        gm = sp.tile([128, 2, N], F32, tag="gm")
        m8b = sp.tile([128, 2, 8], F32, tag="m8b")
        i8b = sp.tile([128, 2, 8], U32, tag="i8b")
        adj = sp.tile([128, 2, N], BF16, tag="adj")
        for i, ni in enumerate(NT):
            nc.vector.max(m8[:ni, i, :], gs[:ni, i, :])
            nc.vector.max_index(i9[:ni, i, 0:8], m8[:ni, i, :], gs[:ni, i, :])
            nc.vector.match_replace(gm[:ni, i, :], m8[:ni, i, :], gs[:ni, i, :], NEG)
            nc.vector.max(m8b[:ni, i, :], gm[:ni, i, :])
            nc.vector.max_index(i8b[:ni, i, :], m8b[:ni, i, :], gm[:ni, i, :])
            nc.vector.tensor_copy(i9[:ni, i, 8:9], i8b[:ni, i, 0:1])
            nc.vector.tensor_scalar(
                adj[:ni, i, :], gs[:ni, i, :], m8b[:ni, i, 0:1], None, op0=ALU.is_ge,
            )
        pc[("i9", s2)] = i9
        pc[("adj", s2)] = adj

        # wrapped int16 index list: j = 256k + n -> (n,k) at out[n%128, 2k+n//128]
        didx = dp.tile([256, K], U32, tag="didx")
        nc.scalar.dma_start(didx[208:256, :], self.z32[:, :])
        nc.scalar.dma_start(didx[0:128, :], i9[:, 0, :])
        nc.scalar.dma_start(didx[128:210, :], i9[:82, 1, :])
        idxw = sp.tile([128, 3, 48], I16, tag="idxw")
        nc.vector.memset(idxw[:, :, :], 0)
        src = didx[:, :].bitcast(I16).rearrange(
            "(b p) (t a two) -> p t a b two", b=16, p=16, t=3, a=3, two=2)
        dst = idxw[0:16, :, :].rearrange("p t (a b) -> p t a b", a=3, b=16)
        nc.scalar.dma_start(dst, src[:, :, :, :, 0])
        # Q7 cores read their own 16-partition stripe: replicate (log2 doubling)
        for g in (16, 32, 64):
            nc.scalar.dma_start(idxw[g:2 * g, :, :], idxw[0:g, :, :])

        # A, B edge-conv halves (bf16), B -> DRAM for the gather
        ABp = sp.tile([128, 2, 2, C2], BF16, tag="ABp")
        bvd = dp.tile([N, C2], BF16, tag="bvd")
        for i, ni in enumerate(NT):
            for hf in range(2):
                ps = self.pab.tile([128, 2, 512], F32, tag="ab")
                for j in range(CCH):
                    lhs = hbb[:, j, s2, i * 128:i * 128 + ni]
                    nc.tensor.matmul(
                        out=ps[:ni, 0, 0:384], lhsT=lhs,
                        rhs=self.wat[:, j, hf * 384:(hf + 1) * 384],
                        start=(j == 0), stop=(j == CCH - 1),
                    )
                    nc.tensor.matmul(
                        out=ps[:ni, 1, 0:384], lhsT=lhs,
                        rhs=self.wbt[:, j, hf * 384:(hf + 1) * 384],
                        start=(j == 0), stop=(j == CCH - 1),
                    )
                nc.scalar.activation(
                    ABp[:ni, :, i, hf * 384:(hf + 1) * 384], ps[:ni, :, 0:384],
                    AF.Copy)
            nc.sync.dma_start(bvd[i * 128:i * 128 + ni, :], ABp[:ni, 1, i, :])
        pc[("ABp", s2)] = ABp

        # per-t gather + incremental max: tree overlaps the gather DMAs
        amax = sp.tile([128, 2, C2], BF16, tag="amax")
        for t in range(3):
            nidx = 722 if t == 2 else 768
            ns = (nidx + 15) // 16
            gt = self.gtp.tile([128, 6, C2], BF16, tag="gt")
            nc.gpsimd.dma_gather(
                out_ap=gt[:, :, :], in_ap=bvd[:, :],
                idxs_ap=idxw[:, t, :ns], num_idxs=nidx, num_idxs_reg=nidx,
                elem_size=C2,
            )
            if t == 0:
                nc.vector.tensor_tensor(out=amax[:, :, :], in0=gt[:, 0:2, :],
                                        in1=gt[:, 2:4, :], op=ALU.max)
                nc.vector.tensor_tensor(out=amax[:, :, :], in0=amax[:, :, :],
                                        in1=gt[:, 4:6, :], op=ALU.max)
            else:
                nc.vector.tensor_tensor(out=gt[:, 0:2, :], in0=gt[:, 0:2, :],
                                        in1=gt[:, 2:4, :], op=ALU.max)
                nc.vector.tensor_tensor(out=gt[:, 0:2, :], in0=gt[:, 0:2, :],
                                        in1=gt[:, 4:6, :], op=ALU.max)
                nc.vector.tensor_tensor(out=amax[:, :, :], in0=amax[:, :, :],
                                        in1=gt[:, 0:2, :], op=ALU.max)
        pc[("amax", s2)] = amax

    # ---- stage B2: max tree -> amT -> reluT; adjT/lrT -> lmp ----
    def stage_b2(self, s):
        pr, s2 = s
        nc, sp = self.nc, self.sp
        pc = self.pc[pr]
        amax, ABp = pc[("amax", s2)], pc[("ABp", s2)]
        i9, adj = pc[("i9", s2)], pc[("adj", s2)]
        reluT, lmp = pc["reluT"], pc["lmp"]
        lrb = pc["lrb"]

        am = sp.tile([128, 2, C2], BF16, tag="am")
        nc.vector.tensor_add(am[:, :, :], ABp[:, 0, :, :], amax[:, :, :])

        # transpose am -> [c, n]; relu(+shift_e) -> reluT
        for cc in range(C2CH):
            pt = self.pab.tile([128, N], BF16, tag="tr")
            for i, ni in enumerate(NT):
                nc.tensor.transpose(
                    pt[:, i * 128:i * 128 + ni], am[:ni, i, cc * 128:(cc + 1) * 128],
                    self.identb[:ni, :ni])
            nc.scalar.activation(
                reluT[:, cc, s2, :], pt[:, :], AF.Relu,
                bias=self.shifte[:, cc:cc + 1],
            )

        # lr^T and Adj^T (bf16), lr_mean = (lr @ Adj^T)/9
        lrT = sp.tile([128, 2, R], BF16, tag="lrT")
        adjT = sp.tile([128, 2, N], BF16, tag="adjT")
        for i, ni in enumerate(NT):
            pt = self.pab.tile([128, N], BF16, tag="tr")
            nc.tensor.transpose(
                pt[:ni, :R], lrb[:, s2, i * 128:i * 128 + ni], self.identb[:R, :R])
            nc.scalar.activation(lrT[:ni, i, :], pt[:ni, :R], AF.Copy)
        for io, nio in enumerate(NT):
            pt = self.pab.tile([128, N], BF16, tag="tr")
            for ii, nii in enumerate(NT):
                nc.tensor.transpose(
                    pt[:nio, ii * 128:ii * 128 + nii],
                    adj[:nii, ii, io * 128:io * 128 + nio],
                    self.identb[:nii, :nii],
                )
            nc.scalar.activation(adjT[:nio, io, :], pt[:nio, :], AF.Copy)

        pslm = self.pmm.tile([R, N], F32, tag="mm")
        for i, ni in enumerate(NT):
            nc.tensor.matmul(
                out=pslm[:, :], lhsT=lrT[:ni, i, :], rhs=adjT[:ni, i, :],
                start=(i == 0), stop=(i == 1),
            )
        nc.scalar.activation(lmp[:, s2, :], pslm[:, :], AF.Copy, scale=1.0 / 9.0)

    # ---- stage C: fc2 + ep, residual, store ----
    def stage_c(self, pr, q):
        nc, d = self.nc, self.d
        pc = self.pc[pr]
        reluT, lmp, xp = pc["reluT"], pc["lmp"], pc["xp"]
        for jo in range(CCH):
            ps = self.pmm.tile([128, 2, N], F32, tag="mm")
            for jc in range(C2CH):
                nc.tensor.matmul(
                    out=ps[:, :, :], lhsT=self.wfc2t[:, jc, jo * 128:(jo + 1) * 128],
                    rhs=reluT[:, jc, :, :], start=(jc == 0), stop=False,
                )
            nc.tensor.matmul(out=ps[:, :, :],
                             lhsT=self.wupt[:, jo * 128:(jo + 1) * 128],
                             rhs=lmp[:, :, :], start=False, stop=True)
            tf = self.sp.tile([128, 2, HW], F32, tag="tf")
            nc.scalar.activation(tf[:, :, :], ps[:, :, :HW], AF.Identity,
                                 bias=self.shifto[:, jo:jo + 1])
            yo = self.sp.tile([128, 2, HW], F32, tag="yo")
            nc.vector.tensor_add(yo[:, :, :], tf[:, :, :], xp[:, jo, :, :])
            nc.sync.dma_start(d["y_d"][pr, :, jo, :, :], yo[:, :, :])


# ======================= host side =======================

def _prep_inputs(inputs):
    f32 = np.float32
    bf = ml_dtypes.bfloat16
    s1 = (inputs["bn1_g"] / np.sqrt(inputs["bn1_v"] + EPS)).astype(f32)
    Wfc1 = (inputs["w_fc1"] * s1[:, None]).astype(f32)
    b1 = ((inputs["b_fc1"] - inputs["bn1_m"]) * s1 + inputs["bn1_b"]).astype(f32)
    se = (inputs["bne_g"] / np.sqrt(inputs["bne_v"] + EPS)).astype(f32)
    W1 = inputs["w_ec"][:, :C]
    W2 = inputs["w_ec"][:, C:]
    WA = ((W1 - W2) * se[:, None]).astype(f32)
    WB = (W2 * se[:, None]).astype(f32)
    shift_e = ((inputs["b_ec"] - inputs["bne_m"]) * se + inputs["bne_b"]).astype(f32)
    s2 = (inputs["bn2_g"] / np.sqrt(inputs["bn2_v"] + EPS)).astype(f32)
    Wfc2 = (0.8 * inputs["w_fc2"] * s2[:, None]).astype(f32)
    wup = (0.2 * inputs["w_up"]).astype(f32)
    shift_out = (0.8 * ((inputs["b_fc2"] - inputs["bn2_m"]) * s2 + inputs["bn2_b"])
                 + 0.2 * inputs["b_up"]).astype(f32)

    def chunk_pj(a, nch):  # [nch*128, ...] -> [128, nch, ...]
        return np.ascontiguousarray(
            a.reshape(nch, 128, *a.shape[1:]).transpose(1, 0, *range(2, a.ndim + 1)))

    w = {
        "wfc1t": chunk_pj(Wfc1.T.copy(), CCH),                  # [128,3,384]
        "bias1": chunk_pj(b1, CCH),                             # [128,3]
        "prom": chunk_pj(inputs["node_prompts"].astype(f32), CCH),
        "wdownt": chunk_pj(inputs["w_down"].T.astype(f32).copy(), CCH),
        "bdown": inputs["b_down"].astype(f32).reshape(R, 1),
        "gp": (0.2 * inputs["graph_prompt"]).astype(f32),       # [32,384]
        "wat": chunk_pj(WA.T.copy(), CCH).astype(bf),           # [128,3,768]
        "wbt": chunk_pj(WB.T.copy(), CCH).astype(bf),
        "shifte": chunk_pj(shift_e, C2CH),                      # [128,6]
        "wfc2t": chunk_pj(Wfc2.T.copy(), C2CH).astype(bf),      # [128,6,384]
        "wupt": wup.T.copy().astype(bf),                        # [32,384]
        "shifto": chunk_pj(shift_out, CCH),                     # [128,3]
    }
    w = {k: np.ascontiguousarray(v) for k, v in w.items()}
    return w


def _shard_x(x):
    # -> per-core [NPAIRS, 128, CCH, 2, HW] f32
    shards = []
    for c in range(NCORES):
        xs = x[c * SPC:(c + 1) * SPC].reshape(SPC, C, HW)
        xs = xs.reshape(NPAIRS, 2, CCH, 128, HW).transpose(0, 3, 2, 1, 4)
        shards.append(np.ascontiguousarray(xs.astype(np.float32)))
    return shards


def _unshard_y(results):
    out = np.empty((B, C, H, W), np.float32)
    for c in range(NCORES):
        y = results[c]["y_d"]  # [NPAIRS,128,CCH,2,HW]
        ys = y.transpose(0, 3, 2, 1, 4).reshape(SPC, C, H, W)
        out[c * SPC:(c + 1) * SPC] = ys
    return out


def get_program():
    if "nc" not in _CACHE:
        _CACHE["nc"] = _build_program()
    return _CACHE["nc"]


def run(inputs, trace=False, **kw):
    from concourse.bass_utils import run_bass_kernel_spmd
    nc = get_program()
    w = _prep_inputs(inputs)
    shards = _shard_x(np.asarray(inputs["x"], np.float32))
    in_maps = [{**w, "x_d": shards[c]} for c in range(NCORES)]
    res = run_bass_kernel_spmd(nc, in_maps, list(range(NCORES)), trace=trace, **kw)
    return _unshard_y(res.results), res


def kernel(**inputs):
    y, _ = run(inputs)
    return y


if __name__ == "__main__":
    get_program()
    print("program built OK")


# revision 19
# speedup vs baseline: 1.3370x; 1.1706x over previous
"""Trainium2 Bass kernel for nn_Grapher (GNN message passing block).

Strategy: pure data-parallel over batch B=64 -> 8 cores x 8 samples.
Per sample the edge conv collapses algebraically:
  max_k relu(BN(W_ec @ [x_i; x_j - x_i]))
    = relu(A[:,n] + max_k B[:,idx[n,k]] + shift)
with A = (W1-W2)*se @ h, B = W2*se @ h.  The KNN runs on a 210x210
cosine matrix via vector-engine max/max_index/match_replace.  The
9-neighbor gather of B rows goes through DRAM with InstDMAGatherAnt
(3 calls/sample, wrapped int16 indices replicated across the 8 Q7
cores); index order j = 256k + n lands row (n,k) at out[n%128,
2k + n//128, :] so the max tree runs on strided slices.  Mean-over-K
of the LoRA edge prompts commutes with the 1x1 conv and uses an
adjacency one-hot matmul.  BN scales/shifts are folded on the host.

Pipeline: 2 blocks x 2 pairs; stage A (fc1/lora/blend/norms) batched
per block so Gelu/Sqrt activation-table loads cluster; stage B skewed
(B1 = gram/top9/idx/AB/gathers, B2 = tree/transposes) to hide gather
DMA latency; stage C (fc2+ep) per pair.  fc1/lora/blend run in f32r
(1 PE pass); gram/norms stay f32 to keep the KNN ranking exact.
"""

import sys
from contextlib import ExitStack

import numpy as np

sys.path.insert(0, "/opt/trn_rl_repo")

import ml_dtypes  # noqa: E402
import concourse.bass as bass  # noqa: E402
import concourse.bacc as bacc  # noqa: E402
import concourse.mybir as mybir  # noqa: E402
import concourse.tile as tile  # noqa: E402
from concourse import library_config  # noqa: E402
from concourse.masks import make_identity  # noqa: E402

F32 = mybir.dt.float32
F32R = mybir.dt.float32r
USE_F32R = False
BF16 = mybir.dt.bfloat16
U32 = mybir.dt.uint32
I16 = mybir.dt.int16
AF = mybir.ActivationFunctionType
ALU = mybir.AluOpType

B, C, H, W = 64, 384, 14, 14
R, P, K = 32, 14, 9
H1, N = 15, 210
HW = H * W          # 196
EPS = 1e-5
NCORES = 8
SPC = B // NCORES   # samples per core = 8
NPAIRS = SPC // 2   # 4
CCH = C // 128      # 3 c-chunks
C2 = 2 * C          # 768
C2CH = C2 // 128    # 6
NT = (128, 82)      # node chunks: 210 = 128 + 82
NEG = -1.0e30
GELU_AF = AF.Gelu

_CACHE = {}


def _maybe_r(ap):
    return ap.bitcast(F32R) if USE_F32R else ap


def _build_nc():
    nc = bacc.Bacc(
        "TRN2", target_bir_lowering=False, debug=False,
        enable_asserts=False, num_devices=NCORES,
    )
    d = {}
    di = {
        "x_d": ([NPAIRS, 128, CCH, 2, HW], F32),
        "xb_d": ([NPAIRS, 128, CCH, 2, HW], BF16),
        "wfc1t": ([128, CCH, C], BF16),
        "bias1": ([128, CCH], F32),
        "prom": ([128, CCH, P], F32),
        "wdownt": ([128, CCH, R], BF16),
        "bdown": ([R, 1], F32),
        "gp": ([R, C], BF16),
        "wat": ([128, CCH, C2], BF16),
        "wbt": ([128, CCH, C2], BF16),
        "shifte": ([128, C2CH], F32),
        "wfc2t": ([128, C2CH, C], BF16),
        "wupt": ([R, C], BF16),
        "shifto": ([128, CCH], F32),
    }
    for name, (shape, dt) in di.items():
        d[name] = nc.dram_tensor(name, shape, dt, kind="ExternalInput").ap()
    d["y_d"] = nc.dram_tensor(
        "y_d", [NPAIRS, 128, CCH, 2, HW], F32, kind="ExternalOutput"
    ).ap()
    return nc, d


def _build_program():
    nc, d = _build_nc()
    with tile.TileContext(nc) as tc:
        with ExitStack() as ctx:
            Emitter(ctx, tc, nc, d).emit()
    nc.compile()
    return nc


class Emitter:
    def __init__(self, ctx, tc, nc, d):
        self.ctx, self.tc, self.nc, self.d = ctx, tc, nc, d
        self.pc = {}   # per-pair tile context: pc[pair] = dict

    def emit(self):
        ctx, tc, nc, d = self.ctx, self.tc, self.nc, self.d
        nc.gpsimd.load_library(library_config.mlp)
        self.wp = ctx.enter_context(tc.tile_pool(name="weights", bufs=1))
        self.pa = ctx.enter_context(tc.tile_pool(name="pairp", bufs=1))
        self.hp_pool = ctx.enter_context(tc.tile_pool(name="hptr", bufs=1))
        self.sp = ctx.enter_context(tc.tile_pool(name="samp", bufs=3))
        self.gtp = ctx.enter_context(tc.tile_pool(name="gtp", bufs=3))
        self.pmm = ctx.enter_context(tc.tile_pool(name="pmm", bufs=2, space="PSUM"))
        self.pab = ctx.enter_context(tc.tile_pool(name="pab", bufs=2, space="PSUM"))
        self.dp = ctx.enter_context(tc.tile_pool(name="dscratch", bufs=3, space="DRAM"))

        wp = self.wp

        def wload(name, shape, dt):
            t = wp.tile(shape, dt, name=name)
            nc.sync.dma_start(t[:], d[name])
            return t

        self.wfc1t = wload("wfc1t", [128, CCH, C], BF16)
        self.bias1 = wload("bias1", [128, CCH], F32)
        self.prom = wload("prom", [128, CCH, P], F32)
        self.wdownt = wload("wdownt", [128, CCH, R], BF16)
        self.bdown = wload("bdown", [R, 1], F32)
        self.gp = wload("gp", [R, C], BF16)
        self.wat = wload("wat", [128, CCH, C2], BF16)
        self.wbt = wload("wbt", [128, CCH, C2], BF16)
        self.shifte = wload("shifte", [128, C2CH], F32)
        self.wfc2t = wload("wfc2t", [128, C2CH, C], BF16)
        self.wupt = wload("wupt", [R, C], BF16)
        self.shifto = wload("shifto", [128, CCH], F32)

        identf = wp.tile([128, 128], F32, name="identf")
        make_identity(nc, identf[:, :])
        self.identf = identf
        self.identb = wp.tile([128, 128], BF16, name="identb")
        nc.vector.tensor_copy(self.identb[:, :], identf[:, :])
        self.id08 = wp.tile([128, 128], BF16, name="id08")
        nc.vector.tensor_scalar_mul(self.id08[:, :], self.identb[:, :], 0.8)
        self.ones = wp.tile([128, 1], BF16, name="ones")
        nc.vector.memset(self.ones[:, :], 1.0)
        self.z32 = wp.tile([48, K], U32, name="z32")
        nc.vector.memset(self.z32[:, :], 0)

        for blk in range(2):
            pairs = (2 * blk, 2 * blk + 1)
            for q, pr in enumerate(pairs):
                self.stage_a1(pr, q)
            for q, pr in enumerate(pairs):
                self.stage_a2(pr, q)
            samples = [(pairs[0], 0), (pairs[0], 1), (pairs[1], 0), (pairs[1], 1)]
            for si, s in enumerate(samples):
                self.stage_b1(s)
                if si >= 2:
                    self.stage_b2(samples[si - 2])
            self.stage_b2(samples[2])
            self.stage_b2(samples[3])
            for q, pr in enumerate(pairs):
                self.stage_c(pr, q)

    # ---- stage A1: fc1 + lora(Gelu) + blend ----
    def stage_a1(self, pr, q):
        nc, d = self.nc, self.d
        pc = self.pc[pr] = {}
        xp = self.pa.tile([128, CCH, 2, HW], F32, tag=f"xp{q}")
        nc.sync.dma_start(xp[:], d["x_d"][pr])
        pc["xp"] = xp
        xb = self.pa.tile([128, CCH, 2, HW], BF16, tag=f"xb{q}")
        nc.sync.dma_start(xb[:], d["xb_d"][pr])

        hp = self.hp_pool.tile([128, CCH, 2, N], BF16, tag=f"hp{q}")
        for jo in range(CCH):
            ps = self.pmm.tile([128, 2, HW], F32, tag="mm")
            for ji in range(CCH):
                nc.tensor.matmul(
                    out=ps[:, :, :],
                    lhsT=self.wfc1t[:, ji, jo * 128:(jo + 1) * 128],
                    rhs=xb[:, ji, :, :],
                    start=(ji == 0), stop=(ji == CCH - 1),
                )
            for s2 in range(2):
                nc.scalar.activation(
                    hp[:, jo, s2, :HW], ps[:, s2, :], AF.Identity,
                    bias=self.bias1[:, jo:jo + 1],
                )
        for s2 in range(2):
            nc.scalar.activation(hp[:, :, s2, HW:N], self.prom[:, :, :], AF.Copy)

        lrp = self.pa.tile([R, 2, N], BF16, tag=f"lrp{q}")
        psl = self.pmm.tile([R, 2, N], F32, tag="mm")
        for ji in range(CCH):
            nc.tensor.matmul(
                out=psl[:, :, :], lhsT=self.wdownt[:, ji, :],
                rhs=hp[:, ji, :, :],
                start=(ji == 0), stop=(ji == CCH - 1),
            )
        nc.scalar.activation(lrp[:, :, :], psl[:, :, :], GELU_AF,
                             bias=self.bdown[:, 0:1])
        pc["lrp"] = lrp
        pc["lrb"] = lrp

        hbb = self.pa.tile([128, CCH, 2, N], BF16, tag=f"hbb{q}")
        for jo in range(CCH):
            ps = self.pmm.tile([128, 2, N], F32, tag="mm")
            nc.tensor.matmul(out=ps[:, :, :],
                             lhsT=self.gp[:, jo * 128:(jo + 1) * 128],
                             rhs=lrp[:, :, :], start=True, stop=False)
            nc.tensor.matmul(out=ps[:, :, :], lhsT=self.id08[:, :],
                             rhs=hp[:, jo, :, :],
                             start=False, stop=True)
            nc.scalar.activation(hbb[:, jo, :, :], ps[:, :, :], AF.Copy)
        pc["hbp"] = hbb
        pc["hbb"] = hbb

        # reluT / lmp tiles persist until stage C
        reluT = self.pa.tile([128, C2CH, 2, N], BF16, tag=f"reluT{q}")
        lmp = self.pa.tile([R, 2, N], BF16, tag=f"lmp{q}")
        pc["reluT"], pc["lmp"] = reluT, lmp

    # ---- stage A2: column norms + cinv (Sqrt clustered per block) ----
    def stage_a2(self, pr, q):
        nc = self.nc
        pc = self.pc[pr]
        hbp = pc["hbp"]
        hsq = self.hp_pool.tile([128, CCH, 2, N], BF16, tag=f"hsq{q}")
        nc.vector.tensor_mul(hsq[:, :, :, :], hbp[:, :, :, :], hbp[:, :, :, :])
        pss = self.pmm.tile([1, 2, N], F32, tag="mm")
        for ji in range(CCH):
            nc.tensor.matmul(out=pss[:, :, :], lhsT=self.ones[:, :],
                             rhs=hsq[:, ji, :, :],
                             start=(ji == 0), stop=(ji == CCH - 1))
        den = self.hp_pool.tile([1, 2, N], F32, tag=f"den{q}")
        nc.scalar.activation(den[:, :, :], pss[:, :, :], AF.Sqrt)
        nc.vector.tensor_scalar_add(den[:, :, :], den[:, :, :], 1e-12)
        cinv = self.pa.tile([1, 2, N], F32, tag=f"cinv{q}")
        nc.vector.reciprocal(cinv[:, :, :], den[:, :, :])
        pc["cinv"] = cinv

    # ---- stage B1: gram -> top9 -> idx chain -> A/B -> bvd -> gathers ----
    def stage_b1(self, s):
        pr, s2 = s
        nc, sp, dp = self.nc, self.sp, self.dp
        pc = self.pc[pr]
        hbp, hbb, cinv = pc["hbp"], pc["hbb"], pc["cinv"]

        cbc = sp.tile([128, N], F32, tag="cbc")
        nc.gpsimd.partition_broadcast(cbc[:, :], cinv[:1, s2, :])
        xn = sp.tile([128, CCH, N], BF16, tag="xn")
        for j in range(CCH):
            nc.vector.tensor_mul(xn[:, j, :], hbp[:, j, s2, :], cbc[:, :])

        # G[n, m] = hb[:,n] . xn[:,m]  (f32: KNN ranking accuracy)
        gs = sp.tile([128, 2, N], F32, tag="gs")
        for i, ni in enumerate(NT):
            ps = self.pmm.tile([128, N], F32, tag="mm")
            for j in range(CCH):
                nc.tensor.matmul(
                    out=ps[:ni, :],
                    lhsT=hbp[:, j, s2, i * 128:i * 128 + ni],
                    rhs=xn[:, j, :],
                    start=(j == 0), stop=(j == CCH - 1),
                )
            nc.vector.tensor_copy(gs[:ni, i, :], ps[:ni, :])

        # top-9 per row: top-8 (max/max_index) + 9th (match_replace)
        m8 = sp.tile([128, 2, 8], F32, tag="m8")
        i9 = sp.tile([128, 2, K], U32, tag="i9")
        gm = sp.tile([128, 2, N], F32, tag="gm")
        m8b = sp.tile([128, 2, 8], F32, tag="m8b")
        i8b = sp.tile([128, 2, 8], U32, tag="i8b")
        adj = sp.tile([128, 2, N], BF16, tag="adj")
        for i, ni in enumerate(NT):
            nc.vector.max(m8[:ni, i, :], gs[:ni, i, :])
            nc.vector.max_index(i9[:ni, i, 0:8], m8[:ni, i, :], gs[:ni, i, :])
            nc.vector.match_replace(gm[:ni, i, :], m8[:ni, i, :], gs[:ni, i, :], NEG)
            nc.vector.max(m8b[:ni, i, :], gm[:ni, i, :])
            nc.vector.max_index(i8b[:ni, i, :], m8b[:ni, i, :], gm[:ni, i, :])
            nc.vector.tensor_copy(i9[:ni, i, 8:9], i8b[:ni, i, 0:1])
            nc.vector.tensor_scalar(
                adj[:ni, i, :], gs[:ni, i, :], m8b[:ni, i, 0:1], None, op0=ALU.is_ge,
            )
        pc[("i9", s2)] = i9
        pc[("adj", s2)] = adj

        # wrapped int16 index list: j = 256k + n -> (n,k) at out[n%128, 2k+n//128]
        didx = dp.tile([256, K], U32, tag="didx")
        nc.scalar.dma_start(didx[208:256, :], self.z32[:, :])
        nc.scalar.dma_start(didx[0:128, :], i9[:, 0, :])
        nc.scalar.dma_start(didx[128:210, :], i9[:82, 1, :])
        idxw = sp.tile([128, 3, 48], I16, tag="idxw")
        nc.vector.memset(idxw[:, :, :], 0)
        src = didx[:, :].bitcast(I16).rearrange(
            "(b p) (t a two) -> p t a b two", b=16, p=16, t=3, a=3, two=2)
        dst = idxw[0:16, :, :].rearrange("p t (a b) -> p t a b", a=3, b=16)
        nc.scalar.dma_start(dst, src[:, :, :, :, 0])
        # Q7 cores read their own 16-partition stripe: replicate (log2 doubling)
        for g in (16, 32, 64):
            nc.scalar.dma_start(idxw[g:2 * g, :, :], idxw[0:g, :, :])

        # A, B edge-conv halves (bf16), B -> DRAM for the gather
        ABp = sp.tile([128, 2, 2, C2], BF16, tag="ABp")
        bvd = dp.tile([N, C2], BF16, tag="bvd")
        for i, ni in enumerate(NT):
            for hf in range(2):
                ps = self.pab.tile([128, 2, 512], F32, tag="ab")
                for j in range(CCH):
                    lhs = hbb[:, j, s2, i * 128:i * 128 + ni]
                    nc.tensor.matmul(
                        out=ps[:ni, 0, 0:384], lhsT=lhs,
                        rhs=self.wat[:, j, hf * 384:(hf + 1) * 384],
                        start=(j == 0), stop=(j == CCH - 1),
                    )
                    nc.tensor.matmul(
                        out=ps[:ni, 1, 0:384], lhsT=lhs,
                        rhs=self.wbt[:, j, hf * 384:(hf + 1) * 384],
                        start=(j == 0), stop=(j == CCH - 1),
                    )
                nc.scalar.activation(
                    ABp[:ni, :, i, hf * 384:(hf + 1) * 384], ps[:ni, :, 0:384],
                    AF.Copy)
            nc.sync.dma_start(bvd[i * 128:i * 128 + ni, :], ABp[:ni, 1, i, :])
        pc[("ABp", s2)] = ABp

        # per-t gather + incremental max: tree overlaps the gather DMAs
        amax = sp.tile([128, 2, C2], BF16, tag="amax")
        for t in range(3):
            nidx = 722 if t == 2 else 768
            ns = (nidx + 15) // 16
            gt = self.gtp.tile([128, 6, C2], BF16, tag="gt")
            nc.gpsimd.dma_gather(
                out_ap=gt[:, :, :], in_ap=bvd[:, :],
                idxs_ap=idxw[:, t, :ns], num_idxs=nidx, num_idxs_reg=nidx,
                elem_size=C2,
            )
            if t == 0:
                nc.vector.tensor_tensor(out=amax[:, :, :], in0=gt[:, 0:2, :],
                                        in1=gt[:, 2:4, :], op=ALU.max)
                nc.vector.tensor_tensor(out=amax[:, :, :], in0=amax[:, :, :],
                                        in1=gt[:, 4:6, :], op=ALU.max)
            else:
                nc.vector.tensor_tensor(out=gt[:, 0:2, :], in0=gt[:, 0:2, :],
                                        in1=gt[:, 2:4, :], op=ALU.max)
                nc.vector.tensor_tensor(out=gt[:, 0:2, :], in0=gt[:, 0:2, :],
                                        in1=gt[:, 4:6, :], op=ALU.max)
                nc.vector.tensor_tensor(out=amax[:, :, :], in0=amax[:, :, :],
                                        in1=gt[:, 0:2, :], op=ALU.max)
        pc[("amax", s2)] = amax

    # ---- stage B2: max tree -> amT -> reluT; adjT/lrT -> lmp ----
    def stage_b2(self, s):
        pr, s2 = s
        nc, sp = self.nc, self.sp
        pc = self.pc[pr]
        amax, ABp = pc[("amax", s2)], pc[("ABp", s2)]
        i9, adj = pc[("i9", s2)], pc[("adj", s2)]
        reluT, lmp = pc["reluT"], pc["lmp"]
        lrb = pc["lrb"]

        am = sp.tile([128, 2, C2], BF16, tag="am")
        nc.vector.tensor_add(am[:, :, :], ABp[:, 0, :, :], amax[:, :, :])

        # transpose am -> [c, n]; relu(+shift_e) -> reluT
        for cc in range(C2CH):
            pt = self.pab.tile([128, N], BF16, tag="tr")
            for i, ni in enumerate(NT):
                nc.tensor.transpose(
                    pt[:, i * 128:i * 128 + ni], am[:ni, i, cc * 128:(cc + 1) * 128],
                    self.identb[:ni, :ni])
            nc.scalar.activation(
                reluT[:, cc, s2, :], pt[:, :], AF.Relu,
                bias=self.shifte[:, cc:cc + 1],
            )

        # lr^T and Adj^T (bf16), lr_mean = (lr @ Adj^T)/9
        lrT = sp.tile([128, 2, R], BF16, tag="lrT")
        adjT = sp.tile([128, 2, N], BF16, tag="adjT")
        for i, ni in enumerate(NT):
            pt = self.pab.tile([128, N], BF16, tag="tr")
            nc.tensor.transpose(
                pt[:ni, :R], lrb[:, s2, i * 128:i * 128 + ni], self.identb[:R, :R])
            nc.scalar.activation(lrT[:ni, i, :], pt[:ni, :R], AF.Copy)
        for io, nio in enumerate(NT):
            pt = self.pab.tile([128, N], BF16, tag="tr")
            for ii, nii in enumerate(NT):
                nc.tensor.transpose(
                    pt[:nio, ii * 128:ii * 128 + nii],
                    adj[:nii, ii, io * 128:io * 128 + nio],
                    self.identb[:nii, :nii],
                )
            nc.scalar.activation(adjT[:nio, io, :], pt[:nio, :], AF.Copy)

        pslm = self.pmm.tile([R, N], F32, tag="mm")
        for i, ni in enumerate(NT):
            nc.tensor.matmul(
                out=pslm[:, :], lhsT=lrT[:ni, i, :], rhs=adjT[:ni, i, :],
                start=(i == 0), stop=(i == 1),
            )
        nc.scalar.activation(lmp[:, s2, :], pslm[:, :], AF.Copy, scale=1.0 / 9.0)

    # ---- stage C: fc2 + ep, residual, store ----
    def stage_c(self, pr, q):
        nc, d = self.nc, self.d
        pc = self.pc[pr]
        reluT, lmp, xp = pc["reluT"], pc["lmp"], pc["xp"]
        for jo in range(CCH):
            ps = self.pmm.tile([128, 2, N], F32, tag="mm")
            for jc in range(C2CH):
                nc.tensor.matmul(
                    out=ps[:, :, :], lhsT=self.wfc2t[:, jc, jo * 128:(jo + 1) * 128],
                    rhs=reluT[:, jc, :, :], start=(jc == 0), stop=False,
                )
            nc.tensor.matmul(out=ps[:, :, :],
                             lhsT=self.wupt[:, jo * 128:(jo + 1) * 128],
                             rhs=lmp[:, :, :], start=False, stop=True)
            tf = self.sp.tile([128, 2, HW], F32, tag="tf")
            nc.scalar.activation(tf[:, :, :], ps[:, :, :HW], AF.Identity,
                                 bias=self.shifto[:, jo:jo + 1])
            yo = self.sp.tile([128, 2, HW], F32, tag="yo")
            nc.vector.tensor_add(yo[:, :, :], tf[:, :, :], xp[:, jo, :, :])
            nc.sync.dma_start(d["y_d"][pr, :, jo, :, :], yo[:, :, :])


# ======================= host side =======================

def _prep_inputs(inputs):
    f32 = np.float32
    bf = ml_dtypes.bfloat16
    s1 = (inputs["bn1_g"] / np.sqrt(inputs["bn1_v"] + EPS)).astype(f32)
    Wfc1 = (inputs["w_fc1"] * s1[:, None]).astype(f32)
    b1 = ((inputs["b_fc1"] - inputs["bn1_m"]) * s1 + inputs["bn1_b"]).astype(f32)
    se = (inputs["bne_g"] / np.sqrt(inputs["bne_v"] + EPS)).astype(f32)
    W1 = inputs["w_ec"][:, :C]
    W2 = inputs["w_ec"][:, C:]
    WA = ((W1 - W2) * se[:, None]).astype(f32)
    WB = (W2 * se[:, None]).astype(f32)
    shift_e = ((inputs["b_ec"] - inputs["bne_m"]) * se + inputs["bne_b"]).astype(f32)
    s2 = (inputs["bn2_g"] / np.sqrt(inputs["bn2_v"] + EPS)).astype(f32)
    Wfc2 = (0.8 * inputs["w_fc2"] * s2[:, None]).astype(f32)
    wup = (0.2 * inputs["w_up"]).astype(f32)
    shift_out = (0.8 * ((inputs["b_fc2"] - inputs["bn2_m"]) * s2 + inputs["bn2_b"])
                 + 0.2 * inputs["b_up"]).astype(f32)

    def chunk_pj(a, nch):  # [nch*128, ...] -> [128, nch, ...]
        return np.ascontiguousarray(
            a.reshape(nch, 128, *a.shape[1:]).transpose(1, 0, *range(2, a.ndim + 1)))

    w = {
        "wfc1t": chunk_pj(Wfc1.T.copy(), CCH).astype(bf),       # [128,3,384]
        "bias1": chunk_pj(b1, CCH),                             # [128,3]
        "prom": chunk_pj(inputs["node_prompts"].astype(f32), CCH),
        "wdownt": chunk_pj(inputs["w_down"].T.astype(f32).copy(), CCH).astype(bf),
        "bdown": inputs["b_down"].astype(f32).reshape(R, 1),
        "gp": (0.2 * inputs["graph_prompt"]).astype(bf),        # [32,384]
        "wat": chunk_pj(WA.T.copy(), CCH).astype(bf),           # [128,3,768]
        "wbt": chunk_pj(WB.T.copy(), CCH).astype(bf),
        "shifte": chunk_pj(shift_e, C2CH),                      # [128,6]
        "wfc2t": chunk_pj(Wfc2.T.copy(), C2CH).astype(bf),      # [128,6,384]
        "wupt": wup.T.copy().astype(bf),                        # [32,384]
        "shifto": chunk_pj(shift_out, CCH),                     # [128,3]
    }
    w = {k: np.ascontiguousarray(v) for k, v in w.items()}
    return w


def _shard_x(x):
    # -> per-core [NPAIRS, 128, CCH, 2, HW] f32
    shards = []
    for c in range(NCORES):
        xs = x[c * SPC:(c + 1) * SPC].reshape(SPC, C, HW)
        xs = xs.reshape(NPAIRS, 2, CCH, 128, HW).transpose(0, 3, 2, 1, 4)
        shards.append(np.ascontiguousarray(xs.astype(np.float32)))
    return shards


def _unshard_y(results):
    out = np.empty((B, C, H, W), np.float32)
    for c in range(NCORES):
        y = results[c]["y_d"]  # [NPAIRS,128,CCH,2,HW]
        ys = y.transpose(0, 3, 2, 1, 4).reshape(SPC, C, H, W)
        out[c * SPC:(c + 1) * SPC] = ys
    return out


def get_program():
    if "nc" not in _CACHE:
        _CACHE["nc"] = _build_program()
    return _CACHE["nc"]


def run(inputs, trace=False, **kw):
    from concourse.bass_utils import run_bass_kernel_spmd
    nc = get_program()
    w = _prep_inputs(inputs)
    shards = _shard_x(np.asarray(inputs["x"], np.float32))
    import ml_dtypes as _md
    in_maps = [{**w, "x_d": shards[c],
                "xb_d": shards[c].astype(_md.bfloat16)} for c in range(NCORES)]
    res = run_bass_kernel_spmd(nc, in_maps, list(range(NCORES)), trace=trace, **kw)
    return _unshard_y(res.results), res


def kernel(**inputs):
    y, _ = run(inputs)
    return y


if __name__ == "__main__":
    get_program()
    print("program built OK")
